# revision 1
# baseline (speedup 1.0000x reference)
"""Trainium2 Bass kernel for CompositionModel (gnn_message_passing).

Model: per-cell MLP over [log1p(X) ++ Z[cell_to_batch]] followed by a
segment-mean over batch labels.

Strategy:
  * Host: sort cells by segment id, pad each segment run to a multiple of 64
    so every 64-cell "minichunk" is single-segment; gather Z rows per cell;
    ship everything transposed (features on partitions) in bf16, blocked as
    [P, 512]-column blocks; two blocks share one DMA/log1p pass.
  * Device (8 cores, data-parallel over cells, identical static program):
      log1p (ACT Ln, 1024 cols/op) -> L1 matmul (K=128 X-part + K=32 Z-part,
      bf16) -> bias+ReLU -> fp8 h1 -> L2 as fp8 DoubleRow matmuls against
      W2 split into a (hi, lo) fp8 pair sharing one x64 scale (W2 is then
      effectively exact; only h1 carries fp8 rounding, which averages out
      in the segment mean) -> fused bias+ReLU+cast on DVE -> GpSimd
      pre-folds each minichunk in half -> grouped DVE tensor_reduce.
      The third (linear) layer commutes with the segment sum and is applied
    on the host to the 512x256 segment sums instead of 500k cells.
  * Host epilogue: subtract the (identical, analytically known) contribution
    of pad cells, scatter-add minichunk sums into segment sums, undo the x64
    W2 scale, apply W3/b3 and divide by true counts.
"""

import numpy as np
import ml_dtypes

import concourse.bacc as bacc
import concourse.mybir as mybir
import concourse.tile as tile
from concourse.bass_utils import run_bass_kernel_spmd

BF16 = ml_dtypes.bfloat16
FP8 = ml_dtypes.float8_e4m3fn

N_CORES = 8
DX = 128
DZ = 32
H = 256
B = 512
MC = 64            # minichunk: cells per single-segment group
BLK = 512          # cells per device block (matmul moving free dim)
NBLK = 126         # blocks per core (fits the fixed reference input)
W2SCALE = 64.0     # fp8 pre-scale on W2/b2, divided out on the host

_compiled = {}
_last_in_maps = None


def _build_program(nblk):
    f32 = mybir.dt.float32
    bf16 = mybir.dt.bfloat16
    fp8 = mybir.dt.float8e4
    Alu = mybir.AluOpType
    Act = mybir.ActivationFunctionType
    DR = mybir.MatmulPerfMode.DoubleRow
    mc_per_core = nblk * (BLK // MC)

    nc = bacc.Bacc("TRN2", target_bir_lowering=False, debug=False,
                   num_devices=N_CORES)

    xt_d = nc.dram_tensor("xt", [nblk // 2, DX, 2 * BLK], bf16,
                          kind="ExternalInput")
    zct_d = nc.dram_tensor("zct", [nblk, DZ, BLK], bf16, kind="ExternalInput")
    w1x_d = nc.dram_tensor("w1x", [DX, H], bf16, kind="ExternalInput")
    w1z_d = nc.dram_tensor("w1z", [DZ, H], bf16, kind="ExternalInput")
    # [m-half][hi/lo][p, ktile*128] fp8, pre-scaled by W2SCALE
    w2_d = nc.dram_tensor("w2", [2, 2, 128, 2 * 128], fp8,
                          kind="ExternalInput")
    b1_d = nc.dram_tensor("b1", [2, 128, 1], f32, kind="ExternalInput")
    b2_d = nc.dram_tensor("b2", [2, 128, 1], f32, kind="ExternalInput")
    out_d = nc.dram_tensor("out", [128, 2 * mc_per_core], f32,
                           kind="ExternalOutput")

    with tile.TileContext(nc) as tc:
        with tc.tile_pool(name="consts", bufs=1) as cpool, \
             tc.tile_pool(name="work", bufs=4) as pool, \
             tc.tile_pool(name="psum", bufs=2, space="PSUM") as psum:

            w1xa = cpool.tile([DX, 128], bf16, tag="w1xa")
            w1xb = cpool.tile([DX, 128], bf16, tag="w1xb")
            nc.sync.dma_start(w1xa[:], w1x_d[:, 0:128])
            nc.sync.dma_start(w1xb[:], w1x_d[:, 128:256])
            w1za = cpool.tile([DZ, 128], bf16, tag="w1za")
            w1zb = cpool.tile([DZ, 128], bf16, tag="w1zb")
            nc.sync.dma_start(w1za[:], w1z_d[:, 0:128])
            nc.sync.dma_start(w1zb[:], w1z_d[:, 128:256])
            w2t = {}
            for m in range(2):
                for t in range(2):
                    w = cpool.tile([128, 2 * 128], fp8, tag=f"w2_{m}{t}")
                    nc.sync.dma_start(w[:], w2_d[m, t])
                    w2t[m, t] = w[:].rearrange("p (k m) -> p k m", k=2)
            b1a = cpool.tile([128, 1], f32, tag="b1a")
            b1b = cpool.tile([128, 1], f32, tag="b1b")
            b2a = cpool.tile([128, 1], f32, tag="b2a")
            b2b = cpool.tile([128, 1], f32, tag="b2b")
            nc.sync.dma_start(b1a[:], b1_d[0])
            nc.sync.dma_start(b1b[:], b1_d[1])
            nc.sync.dma_start(b2a[:], b2_d[0])
            nc.sync.dma_start(b2b[:], b2_d[1])
            ones = cpool.tile([128, 1], f32, tag="ones")
            nc.vector.memset(ones[:], 1.0)

            out2 = cpool.tile([128, 2 * mc_per_core], f32, tag="out2")

            # two blocks share one DMA + one Ln op (amortize ACT overhead);
            # the Ln is emitted two superblocks ahead so it fills ACT idle
            # time without ever delaying a relu that gates the PE
            def emit_ln(k):
                xt = pool.tile([DX, 2 * BLK], bf16, tag="xt")
                nc.sync.dma_start(xt[:], xt_d[k])
                xl = pool.tile([DX, 2 * BLK], bf16, tag="xl")
                nc.scalar.activation(xl[:], xt[:], Act.Ln, bias=ones[:])
                return xl

            nsb = nblk // 2
            xls_ahead = [emit_ln(0), emit_ln(1) if nsb > 1 else None]
            for sblk in range(nsb):
                xl_cur = xls_ahead.pop(0)
                for half in range(2):
                    blk = 2 * sblk + half
                    xls = xl_cur[:, half * BLK:(half + 1) * BLK]
                    zct = pool.tile([DZ, BLK], bf16, tag="zct")
                    nc.sync.dma_start(zct[:], zct_d[blk])

                    ps1a = psum.tile([128, BLK], f32, tag="ps1a")
                    nc.tensor.matmul(ps1a[:], w1xa[:], xls, start=True, stop=False)
                    nc.tensor.matmul(ps1a[:], w1za[:], zct[:], start=False, stop=True)
                    ps1b = psum.tile([128, BLK], f32, tag="ps1b")
                    nc.tensor.matmul(ps1b[:], w1xb[:], xls, start=True, stop=False)
                    nc.tensor.matmul(ps1b[:], w1zb[:], zct[:], start=False, stop=True)

                    # h1 halves stacked as the two DoubleRow k-tiles, fp8
                    h1 = pool.tile([128, 2 * BLK], fp8, tag="h1")
                    nc.scalar.activation(h1[:, 0:BLK], ps1a[:], Act.Relu,
                                         bias=b1a[:])
                    nc.scalar.activation(h1[:, BLK:2 * BLK], ps1b[:], Act.Relu,
                                         bias=b1b[:])
                    h1v = h1[:].rearrange("p (k c) -> p k c", k=2)

                    # the (2x-scaled) lo-term runs on even blocks only: the
                    # correction is ~3% of scale so 2x-on-half-the-cells is
                    # first-order exact through the relu and the segment mean
                    lo = blk % 2 == 0
                    ps2a = psum.tile([128, BLK], f32, tag="ps2a")
                    nc.tensor.matmul(ps2a[:], w2t[0, 0], h1v, start=True,
                                     stop=not lo, perf_mode=DR)
                    if lo:
                        nc.tensor.matmul(ps2a[:], w2t[0, 1], h1v, start=False,
                                         stop=True, perf_mode=DR)
                    ps2b = psum.tile([128, BLK], f32, tag="ps2b")
                    nc.tensor.matmul(ps2b[:], w2t[1, 0], h1v, start=True,
                                     stop=not lo, perf_mode=DR)
                    if lo:
                        nc.tensor.matmul(ps2b[:], w2t[1, 1], h1v, start=False,
                                         stop=True, perf_mode=DR)

                    h2 = pool.tile([128, 2 * BLK], bf16, tag="h2")
                    nc.vector.tensor_scalar(h2[:, 0:BLK], ps2a[:], b2a[:], 0.0,
                                            op0=Alu.add, op1=Alu.max)
                    nc.vector.tensor_scalar(h2[:, BLK:2 * BLK], ps2b[:], b2b[:],
                                            0.0, op0=Alu.add, op1=Alu.max)

                    # GpSimd pre-folds each 64-cell minichunk in half
                    # (SBUF->SBUF add), halving the DVE reduce read size.
                    h2v = h2[:].rearrange("p (g t m) -> p g t m", t=2, m=MC // 2)
                    h2f = pool.tile([128, BLK], bf16, tag="h2f")
                    h2fv = h2f[:].rearrange("p (g m) -> p g m", m=MC // 2)
                    nc.gpsimd.tensor_tensor(
                        h2fv, h2v[:, :, 0:1, :], h2v[:, :, 1:2, :], op=Alu.add)

                    oslice = slice(blk * 2 * (BLK // MC),
                                   (blk + 1) * 2 * (BLK // MC))
                    nc.vector.tensor_reduce(
                        out2[:, oslice], h2fv,
                        axis=mybir.AxisListType.X, op=Alu.add)
                if sblk + 2 < nsb:
                    xls_ahead.append(emit_ln(sblk + 2))

            nc.sync.dma_start(out_d[:], out2[:])

    nc.compile()
    return nc


def _get_program(nblk):
    if nblk not in _compiled:
        _compiled[nblk] = _build_program(nblk)
    return _compiled[nblk]


def kernel(X, Z, W1, b1, W2, b2, W3, b3, cell_to_batch, sample_idx_batch):
    X = np.asarray(X)
    Z = np.asarray(Z)
    W1 = np.asarray(W1, dtype=np.float32)
    b1 = np.asarray(b1, dtype=np.float32)
    W2 = np.asarray(W2, dtype=np.float32)
    b2 = np.asarray(b2, dtype=np.float32)
    W3 = np.asarray(W3, dtype=np.float32)
    b3 = np.asarray(b3, dtype=np.float32)
    c2b = np.asarray(cell_to_batch).astype(np.int64)
    sib = np.asarray(sample_idx_batch).astype(np.int64)

    n = X.shape[0]
    nseg = sib.shape[0]
    seg = sib[c2b]

    # ---- host layout prep -------------------------------------------------
    order = np.argsort(seg, kind="stable")
    seg_sorted = seg[order]
    counts = np.bincount(seg, minlength=nseg).astype(np.int64)
    padded = ((counts + MC - 1) // MC) * MC
    starts = np.concatenate([[0], np.cumsum(padded)])[:nseg]
    total_pad = int(padded.sum())
    nblk = NBLK
    while total_pad > N_CORES * nblk * BLK:  # safety fallback, recompiles
        nblk += 2
    ntot = N_CORES * nblk * BLK
    mc_per_core = nblk * (BLK // MC)
    run_starts = np.concatenate([[0], np.cumsum(counts)])[:nseg]
    ranks = np.arange(n, dtype=np.int64) - run_starts[seg_sorted]
    slots = starts[seg_sorted] + ranks

    Xs = np.zeros((ntot, DX), dtype=BF16)
    Xs[slots] = X[order].astype(BF16)
    Zs = np.zeros((ntot, DZ), dtype=BF16)
    Zs[slots] = Z[c2b[order]].astype(BF16)

    xt = np.ascontiguousarray(
        Xs.reshape(N_CORES, nblk // 2, 2 * BLK, DX).transpose(0, 1, 3, 2))
    zct = np.ascontiguousarray(
        Zs.reshape(N_CORES, nblk, BLK, DZ).transpose(0, 1, 3, 2))

    n_mc = ntot // MC
    mc_label = np.full(n_mc, -1, dtype=np.int64)
    mc_real = np.zeros(n_mc, dtype=np.int64)
    mc_of_slot = slots // MC
    mc_label[mc_of_slot] = seg_sorted
    np.add.at(mc_real, mc_of_slot, 1)

    # ---- weights ----------------------------------------------------------
    w1x = np.ascontiguousarray(W1[:DX]).astype(BF16)
    w1z = np.ascontiguousarray(W1[DX:DX + DZ]).astype(BF16)
    # W2 as a scaled fp8 (hi, lo) pair; together they are W2 to ~4e-4
    w2f = W2.astype(BF16).astype(np.float32) * W2SCALE
    t_hi = w2f.astype(FP8)
    # lo term ships pre-doubled: it is applied on even blocks only
    t_lo = (2.0 * (w2f - t_hi.astype(np.float32))).astype(FP8)
    w2q = np.zeros((2, 2, 128, 2 * 128), dtype=FP8)
    for m in range(2):
        for t, term in enumerate((t_hi, t_lo)):
            # [p, ktile*128] with element [p, k*128+mc] = term[k*128+p, m*128+mc]
            w2q[m, t] = (term.reshape(2, 128, H).transpose(1, 0, 2)
                         [:, :, m * 128:(m + 1) * 128].reshape(128, 256))
    b1d = np.ascontiguousarray(b1.reshape(2, 128, 1))
    b2d = np.ascontiguousarray(b2.reshape(2, 128, 1)) * W2SCALE

    # ---- run on 8 cores ---------------------------------------------------
    nc = _get_program(nblk)
    in_maps = []
    for c in range(N_CORES):
        in_maps.append({
            "xt": xt[c], "zct": zct[c],
            "w1x": w1x, "w1z": w1z, "w2": w2q, "b1": b1d, "b2": b2d,
        })
    global _last_in_maps
    _last_in_maps = in_maps
    res = run_bass_kernel_spmd(nc, in_maps, list(range(N_CORES)))

    # ---- host epilogue ----------------------------------------------------
    per_core = []
    for c in range(N_CORES):
        o = res.results[c]["out"].reshape(128, nblk, 2, BLK // MC)
        per_core.append(np.concatenate(
            [o[:, :, 0, :].reshape(128, mc_per_core),
             o[:, :, 1, :].reshape(128, mc_per_core)], axis=0))
    sums = np.concatenate(per_core, axis=1)  # [256, n_mc], scaled by W2SCALE

    # analytic contribution of one pad cell (X=0, Z=0), matching device math;
    # even blocks include the doubled lo-term, odd blocks are hi-only
    h1p = np.maximum(b1, 0.0).astype(FP8).astype(np.float32)
    w2eff = t_hi.astype(np.float32) + t_lo.astype(np.float32)
    v_pad_even = np.maximum(h1p @ w2eff + W2SCALE * b2, 0.0) \
        .astype(BF16).astype(np.float32)
    v_pad_odd = np.maximum(h1p @ t_hi.astype(np.float32) + W2SCALE * b2, 0.0) \
        .astype(BF16).astype(np.float32)
    mc_parity = (np.arange(n_mc) // (BLK // MC)) % 2
    v_pad = np.where(mc_parity[None, :] == 0,
                     v_pad_even[:, None], v_pad_odd[:, None])
    sums = sums - v_pad * (MC - mc_real).astype(np.float32)[None, :]
    sums /= W2SCALE

    valid = mc_label >= 0
    S = np.zeros((nseg, H), dtype=np.float32)
    np.add.at(S, mc_label[valid], sums[:, valid].T)

    denom = np.maximum(counts, 1).astype(np.float32)[:, None]
    Y = S @ W3 / denom + b3[None, :]
    Y[counts == 0] = 0.0
    return Y.astype(np.float32)



# revision 2
# speedup vs baseline: 1.1005x; 1.1005x over previous
"""Trainium2 Bass kernel for CompositionModel (gnn_message_passing).

Model: per-cell MLP over [log1p(X) ++ Z[cell_to_batch]] followed by a
segment-mean over batch labels.

Strategy (v2):
  * Host: sort cells by segment id, pad each segment run to a multiple of 64
    so every 64-cell "minichunk" is single-segment; apply log1p on the host;
    ship X' transposed (features on partitions) in bf16 as [128, 1024]
    two-block tiles.  The Z covariates never ship per cell: the per-sample
    vector zb1 = Z @ W1z + b1 enters the device matmul as per-block weight
    rows multiplied by a static one-hot "minichunk indicator" operand
    (two concurrent K=8 row-tiled matmuls at partition strips 0 and 32).
  * Device (8 cores, data-parallel over cells, identical static program):
      L1 = W1x^T X' (bf16, K=128, two output halves) + indicator matmuls
      -> ACT relu -> fp8 h1 -> L2 as fp8 DoubleRow matmuls against W2 split
      into a (hi, lo) fp8 pair sharing one x64 scale (lo applied 2x on even
      blocks only; statistically exact through the segment mean) -> DVE
      bias+relu+cast -> GpSimd pairwise folds 64->32->16 -> DVE grouped
      tensor_reduce to per-minichunk sums.
      The PE program is software-pipelined two blocks deep (L1(k) ... L2(k-2))
      so the tensor engine never waits on the ACT relu latency.
      The third (linear) MLP layer commutes with the segment sum and is
      applied on the host to the 512x256 segment sums instead of 500k cells.
  * Host epilogue: subtract the analytically known contribution of pad cells
    (per segment, since pads now carry zb1), scatter-add minichunk sums into
    segment sums, undo the x64 W2 scale, apply W3/b3, divide by true counts.
"""

import numpy as np
import ml_dtypes

import concourse.bacc as bacc
import concourse.mybir as mybir
import concourse.tile as tile
from concourse.bass_utils import run_bass_kernel_spmd

BF16 = ml_dtypes.bfloat16
FP8 = ml_dtypes.float8_e4m3fn

N_CORES = 8
DX = 128
DZ = 32
H = 256
B = 512
MC = 64            # minichunk: cells per single-segment group
BLK = 512          # cells per device block (matmul moving free dim)
NBLK = 126         # blocks per core (fits the fixed reference input)
W2SCALE = 64.0     # fp8 pre-scale on W2/b2, divided out on the host

_compiled = {}
_last_in_maps = None


def _build_program(nblk):
    f32 = mybir.dt.float32
    bf16 = mybir.dt.bfloat16
    fp8 = mybir.dt.float8e4
    Alu = mybir.AluOpType
    Act = mybir.ActivationFunctionType
    DR = mybir.MatmulPerfMode.DoubleRow
    mc_per_core = nblk * (BLK // MC)
    assert nblk % 2 == 0

    nc = bacc.Bacc("TRN2", target_bir_lowering=False, debug=False,
                   num_devices=N_CORES)

    xt_d = nc.dram_tensor("xt", [nblk // 2, DX, 2 * BLK], bf16,
                          kind="ExternalInput")
    # per-block zb1 rows: [half, minichunk(8), nblk*128]
    wind_d = nc.dram_tensor("wind", [2, 8, nblk * 128], bf16,
                            kind="ExternalInput")
    # static minichunk one-hot rows at partition strips 0:8 and 32:40
    xind_d = nc.dram_tensor("xind", [40, BLK], bf16, kind="ExternalInput")
    w1x_d = nc.dram_tensor("w1x", [DX, H], bf16, kind="ExternalInput")
    # [m-half][hi/lo][p, ktile*128] fp8, pre-scaled by W2SCALE
    w2_d = nc.dram_tensor("w2", [2, 2, 128, 2 * 128], fp8,
                          kind="ExternalInput")
    b2_d = nc.dram_tensor("b2", [2, 128, 1], f32, kind="ExternalInput")
    out_d = nc.dram_tensor("out", [128, 2 * mc_per_core], f32,
                           kind="ExternalOutput")

    with tile.TileContext(nc) as tc:
        with tc.tile_pool(name="consts", bufs=1) as cpool, \
             tc.tile_pool(name="work", bufs=4) as pool, \
             tc.tile_pool(name="psum", bufs=2, space="PSUM") as psum:

            w1xa = cpool.tile([DX, 128], bf16, tag="w1xa")
            w1xb = cpool.tile([DX, 128], bf16, tag="w1xb")
            nc.sync.dma_start(w1xa[:], w1x_d[:, 0:128])
            nc.sync.dma_start(w1xb[:], w1x_d[:, 128:256])
            w2t = {}
            for m in range(2):
                for t in range(2):
                    w = cpool.tile([128, 2 * 128], fp8, tag=f"w2_{m}{t}")
                    nc.sync.dma_start(w[:], w2_d[m, t])
                    w2t[m, t] = w[:].rearrange("p (k m) -> p k m", k=2)
            b2a = cpool.tile([128, 1], f32, tag="b2a")
            b2b = cpool.tile([128, 1], f32, tag="b2b")
            nc.sync.dma_start(b2a[:], b2_d[0])
            nc.sync.dma_start(b2b[:], b2_d[1])
            xind = cpool.tile([40, BLK], bf16, tag="xind")
            nc.sync.dma_start(xind[:], xind_d[:])
            wia = cpool.tile([40, nblk * 128], bf16, tag="wia")
            nc.sync.dma_start(wia[0:8, :], wind_d[0])
            nc.sync.dma_start(wia[32:40, :], wind_d[1])

            out2 = cpool.tile([128, 2 * mc_per_core], f32, tag="out2")

            xts, ps1s, h1s, ps2s, h2s = {}, {}, {}, {}, {}

            def dma_xt(sb):
                t = pool.tile([DX, 2 * BLK], bf16, tag="xt")
                nc.sync.dma_start(t[:], xt_d[sb])
                xts[sb] = t

            def l1(k):
                sb, half = divmod(k, 2)
                if half == 0 and sb + 2 < nblk // 2:
                    dma_xt(sb + 2)
                xls = xts[sb][:, half * BLK:(half + 1) * BLK]
                ps1a = psum.tile([128, BLK], f32, tag="ps1a")
                nc.tensor.matmul(ps1a[:], w1xa[:], xls, start=True, stop=False)
                ps1b = psum.tile([128, BLK], f32, tag="ps1b")
                nc.tensor.matmul(ps1b[:], w1xb[:], xls, start=True, stop=False)
                # zb1 enters as two concurrent K=8 row-tiled matmuls
                nc.tensor.matmul(ps1a[:], wia[0:8, k * 128:(k + 1) * 128],
                                 xind[0:8, :], start=False, stop=True)
                nc.tensor.matmul(ps1b[:], wia[32:40, k * 128:(k + 1) * 128],
                                 xind[32:40, :], start=False, stop=True)
                ps1s[k] = (ps1a, ps1b)
                if half == 1:
                    xts.pop(sb, None)

            def h1f(k):
                ps1a, ps1b = ps1s.pop(k)
                h1 = pool.tile([128, 2 * BLK], fp8, tag="h1")
                nc.scalar.activation(h1[:, 0:BLK], ps1a[:], Act.Relu)
                nc.scalar.activation(h1[:, BLK:2 * BLK], ps1b[:], Act.Relu)
                h1s[k] = h1

            def l2(k):
                h1 = h1s.pop(k)
                h1v = h1[:].rearrange("p (t c) -> p t c", t=2)
                lo = k % 2 == 0
                ps2a = psum.tile([128, BLK], f32, tag="ps2a")
                nc.tensor.matmul(ps2a[:], w2t[0, 0], h1v, start=True,
                                 stop=not lo, perf_mode=DR)
                if lo:
                    nc.tensor.matmul(ps2a[:], w2t[0, 1], h1v, start=False,
                                     stop=True, perf_mode=DR)
                ps2b = psum.tile([128, BLK], f32, tag="ps2b")
                nc.tensor.matmul(ps2b[:], w2t[1, 0], h1v, start=True,
                                 stop=not lo, perf_mode=DR)
                if lo:
                    nc.tensor.matmul(ps2b[:], w2t[1, 1], h1v, start=False,
                                     stop=True, perf_mode=DR)
                ps2s[k] = (ps2a, ps2b)

            def h2f(k):
                ps2a, ps2b = ps2s.pop(k)
                h2 = pool.tile([128, 2 * BLK], bf16, tag="h2")
                nc.vector.tensor_scalar(h2[:, 0:BLK], ps2a[:], b2a[:], 0.0,
                                        op0=Alu.add, op1=Alu.max)
                nc.vector.tensor_scalar(h2[:, BLK:2 * BLK], ps2b[:], b2b[:],
                                        0.0, op0=Alu.add, op1=Alu.max)
                h2s[k] = h2

            def red(k):
                h2 = h2s.pop(k)
                h2v = h2[:].rearrange("p (g t m) -> p g t m", t=2, m=MC // 2)
                hf = pool.tile([128, BLK], bf16, tag="hf")
                hfv = hf[:].rearrange("p (g m) -> p g m", m=MC // 2)
                nc.gpsimd.tensor_tensor(
                    hfv, h2v[:, :, 0:1, :], h2v[:, :, 1:2, :], op=Alu.add)
                hfv2 = hf[:].rearrange("p (g t m) -> p g t m", t=2, m=MC // 4)
                hg = pool.tile([128, BLK // 2], bf16, tag="hg")
                hgv = hg[:].rearrange("p (g m) -> p g m", m=MC // 4)
                nc.gpsimd.tensor_tensor(
                    hgv, hfv2[:, :, 0:1, :], hfv2[:, :, 1:2, :], op=Alu.add)
                nc.vector.tensor_reduce(
                    out2[:, k * 16:(k + 1) * 16], hgv,
                    axis=mybir.AxisListType.X, op=Alu.add)

            dma_xt(0)
            if nblk > 2:
                dma_xt(1)
            for k in range(nblk + 2):
                if k < nblk:
                    l1(k)
                if 2 <= k:
                    l2(k - 2)
                if 1 <= k <= nblk:
                    h1f(k - 1)
                if 2 <= k:
                    h2f(k - 2)
                    red(k - 2)

            nc.sync.dma_start(out_d[:], out2[:])

    nc.compile()
    return nc


def _get_program(nblk):
    if nblk not in _compiled:
        _compiled[nblk] = _build_program(nblk)
    return _compiled[nblk]


def kernel(X, Z, W1, b1, W2, b2, W3, b3, cell_to_batch, sample_idx_batch):
    X = np.asarray(X)
    Z = np.asarray(Z)
    W1 = np.asarray(W1, dtype=np.float32)
    b1 = np.asarray(b1, dtype=np.float32)
    W2 = np.asarray(W2, dtype=np.float32)
    b2 = np.asarray(b2, dtype=np.float32)
    W3 = np.asarray(W3, dtype=np.float32)
    b3 = np.asarray(b3, dtype=np.float32)
    c2b = np.asarray(cell_to_batch).astype(np.int64)
    sib = np.asarray(sample_idx_batch).astype(np.int64)

    n = X.shape[0]
    nseg = sib.shape[0]
    seg = sib[c2b]

    # ---- host layout prep -------------------------------------------------
    order = np.argsort(seg, kind="stable")
    seg_sorted = seg[order]
    counts = np.bincount(seg, minlength=nseg).astype(np.int64)
    padded = ((counts + MC - 1) // MC) * MC
    starts = np.concatenate([[0], np.cumsum(padded)])[:nseg]
    total_pad = int(padded.sum())
    nblk = NBLK
    while total_pad > N_CORES * nblk * BLK:  # safety fallback, recompiles
        nblk += 2
    ntot = N_CORES * nblk * BLK
    mc_per_core = nblk * (BLK // MC)
    run_starts = np.concatenate([[0], np.cumsum(counts)])[:nseg]
    ranks = np.arange(n, dtype=np.int64) - run_starts[seg_sorted]
    slots = starts[seg_sorted] + ranks

    Xs = np.zeros((ntot, DX), dtype=BF16)
    Xs[slots] = np.log1p(X[order], dtype=np.float32).astype(BF16)

    xt = np.ascontiguousarray(
        Xs.reshape(N_CORES, nblk // 2, 2 * BLK, DX).transpose(0, 1, 3, 2))

    n_mc = ntot // MC
    mc_label = np.full(n_mc, -1, dtype=np.int64)
    mc_real = np.zeros(n_mc, dtype=np.int64)
    mc_of_slot = slots // MC
    mc_label[mc_of_slot] = seg_sorted
    np.add.at(mc_real, mc_of_slot, 1)

    # ---- weights ----------------------------------------------------------
    w1x = np.ascontiguousarray(W1[:DX]).astype(BF16)
    # per-sample covariate projection, folded with b1; bf16 as shipped
    zb1_bf = (Z.astype(np.float32) @ W1[DX:DX + DZ] + b1).astype(BF16)
    lab = mc_label.reshape(N_CORES, nblk, 8)
    wind = zb1_bf[np.maximum(lab, 0)]              # [C, nblk, 8, 256]
    wind[lab < 0] = 0
    wind = np.ascontiguousarray(
        wind.reshape(N_CORES, nblk, 8, 2, 128)
        .transpose(0, 3, 2, 1, 4)                  # [C, half, g, blk, 128]
        .reshape(N_CORES, 2, 8, nblk * 128))

    xind = np.zeros((40, BLK), dtype=BF16)
    for g in range(BLK // MC):
        xind[g, g * MC:(g + 1) * MC] = 1
        xind[32 + g, g * MC:(g + 1) * MC] = 1

    # W2 as a scaled fp8 (hi, lo) pair; together they are W2 to ~4e-4
    w2f = W2.astype(BF16).astype(np.float32) * W2SCALE
    t_hi = w2f.astype(FP8)
    # lo term ships pre-doubled: it is applied on even blocks only
    t_lo = (2.0 * (w2f - t_hi.astype(np.float32))).astype(FP8)
    w2q = np.zeros((2, 2, 128, 2 * 128), dtype=FP8)
    for m in range(2):
        for t, term in enumerate((t_hi, t_lo)):
            # [p, ktile*128] with element [p, k*128+mc] = term[k*128+p, m*128+mc]
            w2q[m, t] = (term.reshape(2, 128, H).transpose(1, 0, 2)
                         [:, :, m * 128:(m + 1) * 128].reshape(128, 256))
    b2d = np.ascontiguousarray(b2.reshape(2, 128, 1)) * W2SCALE

    # ---- run on 8 cores ---------------------------------------------------
    nc = _get_program(nblk)
    in_maps = []
    for c in range(N_CORES):
        in_maps.append({
            "xt": xt[c], "wind": wind[c], "xind": xind,
            "w1x": w1x, "w2": w2q, "b2": b2d,
        })
    global _last_in_maps
    _last_in_maps = in_maps
    res = run_bass_kernel_spmd(nc, in_maps, list(range(N_CORES)))

    # ---- host epilogue ----------------------------------------------------
    per_core = []
    for c in range(N_CORES):
        o = res.results[c]["out"].reshape(128, nblk, 2, BLK // MC)
        per_core.append(np.concatenate(
            [o[:, :, 0, :].reshape(128, mc_per_core),
             o[:, :, 1, :].reshape(128, mc_per_core)], axis=0))
    sums = np.concatenate(per_core, axis=1)  # [256, n_mc], scaled by W2SCALE

    # analytic contribution of one pad cell (X'=0, zb1 applied), matching
    # device math; even blocks include the doubled lo-term, odd hi-only
    h1p = np.maximum(zb1_bf.astype(np.float32), 0.0) \
        .astype(FP8).astype(np.float32)                      # [B, 256]
    w2eff = t_hi.astype(np.float32) + t_lo.astype(np.float32)
    v_even = np.maximum(h1p @ w2eff + W2SCALE * b2, 0.0) \
        .astype(BF16).astype(np.float32)                     # [B, 256]
    v_odd = np.maximum(h1p @ t_hi.astype(np.float32) + W2SCALE * b2, 0.0) \
        .astype(BF16).astype(np.float32)
    mc_parity = (np.arange(n_mc) // (BLK // MC)) % 2
    npad = MC - mc_real
    fix = (mc_label >= 0) & (npad > 0)
    vp = np.where(mc_parity[fix, None] == 0,
                  v_even[mc_label[fix]], v_odd[mc_label[fix]])
    sums[:, fix] -= (vp * npad[fix, None].astype(np.float32)).T
    sums /= W2SCALE

    valid = mc_label >= 0
    S = np.zeros((nseg, H), dtype=np.float32)
    np.add.at(S, mc_label[valid], sums[:, valid].T)

    denom = np.maximum(counts, 1).astype(np.float32)[:, None]
    Y = S @ W3 / denom + b3[None, :]
    Y[counts == 0] = 0.0
    return Y.astype(np.float32)


# revision 4
# speedup vs baseline: 1.4177x; 1.2882x over previous
"""Trainium2 Bass kernel for CompositionModel (gnn_message_passing).

Model: per-cell MLP over [log1p(X) ++ Z[cell_to_batch]] followed by a
segment-mean over batch labels.

Strategy (v2):
  * Host: sort cells by segment id, pad each segment run to a multiple of 64
    so every 64-cell "minichunk" is single-segment; apply log1p on the host;
    ship X' transposed (features on partitions) in bf16 as [128, 1024]
    two-block tiles.  The Z covariates never ship per cell: the per-sample
    vector zb1 = Z @ W1z + b1 enters the device matmul as per-block weight
    rows multiplied by a static one-hot "minichunk indicator" operand
    (two concurrent K=8 row-tiled matmuls at partition strips 0 and 32).
  * Device (8 cores, data-parallel over cells, identical static program):
      L1 = W1x^T X' (bf16, K=128, two output halves) + indicator matmuls
      -> ACT relu -> fp8 h1 -> L2 as fp8 DoubleRow matmuls against W2 split
      into a (hi, lo) fp8 pair sharing one x64 scale (lo applied 2x on even
      blocks only; statistically exact through the segment mean) -> DVE
      bias+relu+cast -> GpSimd pairwise folds 64->32->16 -> DVE grouped
      tensor_reduce to per-minichunk sums.
      The PE program is software-pipelined two blocks deep (L1(k) ... L2(k-2))
      so the tensor engine never waits on the ACT relu latency.
      The third (linear) MLP layer commutes with the segment sum and is
      applied on the host to the 512x256 segment sums instead of 500k cells.
  * Host epilogue: subtract the analytically known contribution of pad cells
    (per segment, since pads now carry zb1), scatter-add minichunk sums into
    segment sums, undo the x64 W2 scale, apply W3/b3, divide by true counts.
"""

import numpy as np
import ml_dtypes

import concourse.bacc as bacc
import concourse.mybir as mybir
import concourse.tile as tile
from concourse.bass_utils import run_bass_kernel_spmd

BF16 = ml_dtypes.bfloat16
FP8 = ml_dtypes.float8_e4m3fn

N_CORES = 8
DX = 128
DZ = 32
H = 256
B = 512
MC = 64            # minichunk: cells per single-segment group
BLK = 512          # cells per device block (matmul moving free dim)
NBLK = 126         # blocks per core (fits the fixed reference input)
W2SCALE = 64.0     # fp8 pre-scale on W2/b2, divided out on the host

_compiled = {}
_last_in_maps = None


def _build_program(nblk):
    f32 = mybir.dt.float32
    bf16 = mybir.dt.bfloat16
    fp8 = mybir.dt.float8e4
    Alu = mybir.AluOpType
    Act = mybir.ActivationFunctionType
    DR = mybir.MatmulPerfMode.DoubleRow
    mc_per_core = nblk * (BLK // MC)
    assert nblk % 2 == 0

    nc = bacc.Bacc("TRN2", target_bir_lowering=False, debug=False,
                   num_devices=N_CORES)

    xt_d = nc.dram_tensor("xt", [nblk // 2, DX, 2 * BLK], bf16,
                          kind="ExternalInput")
    # per-block zb1 rows: [half, minichunk(8), nblk*128]
    wind_d = nc.dram_tensor("wind", [2, 8, nblk * 128], bf16,
                            kind="ExternalInput")
    # static minichunk one-hot rows at partition strips 0:8 and 32:40
    xind_d = nc.dram_tensor("xind", [40, BLK], bf16, kind="ExternalInput")
    w1x_d = nc.dram_tensor("w1x", [DX, H], bf16, kind="ExternalInput")
    # [m-half][hi/lo][p, ktile*128] fp8, pre-scaled by W2SCALE
    w2_d = nc.dram_tensor("w2", [2, 2, 128, 2 * 128], fp8,
                          kind="ExternalInput")
    b2_d = nc.dram_tensor("b2", [2, 128, 1], f32, kind="ExternalInput")
    out_d = nc.dram_tensor("out", [128, 2 * mc_per_core], f32,
                           kind="ExternalOutput")

    with tile.TileContext(nc) as tc:
        with tc.tile_pool(name="consts", bufs=1) as cpool, \
             tc.tile_pool(name="work", bufs=4) as pool, \
             tc.tile_pool(name="psum", bufs=2, space="PSUM") as psum:

            w1xa = cpool.tile([DX, 128], bf16, tag="w1xa")
            w1xb = cpool.tile([DX, 128], bf16, tag="w1xb")
            nc.sync.dma_start(w1xa[:], w1x_d[:, 0:128])
            nc.sync.dma_start(w1xb[:], w1x_d[:, 128:256])
            w2t = {}
            for m in range(2):
                for t in range(2):
                    w = cpool.tile([128, 2 * 128], fp8, tag=f"w2_{m}{t}")
                    nc.sync.dma_start(w[:], w2_d[m, t])
                    w2t[m, t] = w[:].rearrange("p (k m) -> p k m", k=2)
            b2a = cpool.tile([128, 1], f32, tag="b2a")
            b2b = cpool.tile([128, 1], f32, tag="b2b")
            nc.sync.dma_start(b2a[:], b2_d[0])
            nc.sync.dma_start(b2b[:], b2_d[1])
            xind = cpool.tile([40, BLK], bf16, tag="xind")
            nc.sync.dma_start(xind[:], xind_d[:])
            wia = cpool.tile([40, nblk * 128], bf16, tag="wia")
            nc.sync.dma_start(wia[0:8, :], wind_d[0])
            nc.sync.dma_start(wia[32:40, :], wind_d[1])

            out2 = cpool.tile([128, 2 * mc_per_core], f32, tag="out2")

            xts, ps1s, h1s, ps2s, h2s = {}, {}, {}, {}, {}

            def dma_xt(sb):
                t = pool.tile([DX, 2 * BLK], bf16, tag="xt")
                nc.sync.dma_start(t[:], xt_d[sb])
                xts[sb] = t

            def l1(k):
                sb, half = divmod(k, 2)
                if half == 0 and sb + 2 < nblk // 2:
                    dma_xt(sb + 2)
                xls = xts[sb][:, half * BLK:(half + 1) * BLK]
                # single 2-bank psum tile; the K=8 indicator matmuls sit
                # adjacent in PE order (distinct row groups -> concurrent)
                # while keeping each bank's accumulation group contiguous
                ps1 = psum.tile([128, 2 * BLK], f32, tag="ps1")
                nc.tensor.matmul(ps1[:, 0:BLK], w1xa[:], xls,
                                 start=True, stop=False)
                nc.tensor.matmul(ps1[:, 0:BLK],
                                 wia[0:8, k * 128:(k + 1) * 128],
                                 xind[0:8, :], start=False, stop=True)
                nc.tensor.matmul(ps1[:, BLK:2 * BLK],
                                 wia[32:40, k * 128:(k + 1) * 128],
                                 xind[32:40, :], start=True, stop=False)
                nc.tensor.matmul(ps1[:, BLK:2 * BLK], w1xb[:], xls,
                                 start=False, stop=True)
                ps1s[k] = ps1
                if half == 1:
                    xts.pop(sb, None)

            def h1f(k):
                ps1 = ps1s.pop(k)
                h1 = pool.tile([128, 2 * BLK], fp8, tag="h1")
                nc.scalar.activation(h1[:], ps1[:], Act.Relu)
                h1s[k] = h1

            def l2(k):
                h1 = h1s.pop(k)
                h1v = h1[:].rearrange("p (t c) -> p t c", t=2)
                lo = k % 2 == 0
                ps2a = psum.tile([128, BLK], f32, tag="ps2a")
                nc.tensor.matmul(ps2a[:], w2t[0, 0], h1v, start=True,
                                 stop=not lo, perf_mode=DR)
                if lo:
                    nc.tensor.matmul(ps2a[:], w2t[0, 1], h1v, start=False,
                                     stop=True, perf_mode=DR)
                ps2b = psum.tile([128, BLK], f32, tag="ps2b")
                nc.tensor.matmul(ps2b[:], w2t[1, 0], h1v, start=True,
                                 stop=not lo, perf_mode=DR)
                if lo:
                    nc.tensor.matmul(ps2b[:], w2t[1, 1], h1v, start=False,
                                     stop=True, perf_mode=DR)
                ps2s[k] = (ps2a, ps2b)

            def h2f(k):
                ps2a, ps2b = ps2s.pop(k)
                h2 = pool.tile([128, 2 * BLK], bf16, tag="h2")
                # half a alternates ACT/DVE to balance the two engines
                if k % 2 == 0:
                    nc.scalar.activation(h2[:, 0:BLK], ps2a[:], Act.Relu,
                                         bias=b2a[:])
                else:
                    nc.vector.tensor_scalar(h2[:, 0:BLK], ps2a[:], b2a[:],
                                            0.0, op0=Alu.add, op1=Alu.max)
                nc.vector.tensor_scalar(h2[:, BLK:2 * BLK], ps2b[:], b2b[:],
                                        0.0, op0=Alu.add, op1=Alu.max)
                h2s[k] = h2

            hgs = {}

            def folds(k):
                h2 = h2s.pop(k)
                h2v = h2[:].rearrange("p (g t m) -> p g t m", t=2, m=MC // 2)
                hf = pool.tile([128, BLK], bf16, tag="hf")
                hfv = hf[:].rearrange("p (g m) -> p g m", m=MC // 2)
                nc.gpsimd.tensor_tensor(
                    hfv, h2v[:, :, 0:1, :], h2v[:, :, 1:2, :], op=Alu.add)
                hfv2 = hf[:].rearrange("p (g t m) -> p g t m", t=2, m=MC // 4)
                hg = pool.tile([128, BLK // 2], bf16, tag="hg")
                hgv = hg[:].rearrange("p (g m) -> p g m", m=MC // 4)
                nc.gpsimd.tensor_tensor(
                    hgv, hfv2[:, :, 0:1, :], hfv2[:, :, 1:2, :], op=Alu.add)
                hgs[k] = hgv

            def red(k):
                hgv = hgs.pop(k)
                nc.vector.tensor_reduce(
                    out2[:, k * 16:(k + 1) * 16], hgv,
                    axis=mybir.AxisListType.X, op=Alu.add)

            dma_xt(0)
            if nblk > 2:
                dma_xt(1)
            # every stage's dependencies are >=1 iteration old, so no engine
            # ever head-blocks its FIFO waiting on same-iteration work
            for k in range(nblk + 5):
                if k < nblk:
                    l1(k)
                if 0 <= k - 2 < nblk:
                    l2(k - 2)
                if 0 <= k - 1 < nblk:
                    h1f(k - 1)
                if 0 <= k - 3 < nblk:
                    h2f(k - 3)
                if 0 <= k - 4 < nblk:
                    folds(k - 4)
                if 0 <= k - 5 < nblk:
                    red(k - 5)

            nc.sync.dma_start(out_d[:], out2[:])

    nc.compile()
    return nc


def _get_program(nblk):
    if nblk not in _compiled:
        _compiled[nblk] = _build_program(nblk)
    return _compiled[nblk]


def kernel(X, Z, W1, b1, W2, b2, W3, b3, cell_to_batch, sample_idx_batch):
    X = np.asarray(X)
    Z = np.asarray(Z)
    W1 = np.asarray(W1, dtype=np.float32)
    b1 = np.asarray(b1, dtype=np.float32)
    W2 = np.asarray(W2, dtype=np.float32)
    b2 = np.asarray(b2, dtype=np.float32)
    W3 = np.asarray(W3, dtype=np.float32)
    b3 = np.asarray(b3, dtype=np.float32)
    c2b = np.asarray(cell_to_batch).astype(np.int64)
    sib = np.asarray(sample_idx_batch).astype(np.int64)

    n = X.shape[0]
    nseg = sib.shape[0]
    seg = sib[c2b]

    # ---- host layout prep -------------------------------------------------
    order = np.argsort(seg, kind="stable")
    seg_sorted = seg[order]
    counts = np.bincount(seg, minlength=nseg).astype(np.int64)
    padded = ((counts + MC - 1) // MC) * MC
    starts = np.concatenate([[0], np.cumsum(padded)])[:nseg]
    total_pad = int(padded.sum())
    nblk = NBLK
    while total_pad > N_CORES * nblk * BLK:  # safety fallback, recompiles
        nblk += 2
    ntot = N_CORES * nblk * BLK
    mc_per_core = nblk * (BLK // MC)
    run_starts = np.concatenate([[0], np.cumsum(counts)])[:nseg]
    ranks = np.arange(n, dtype=np.int64) - run_starts[seg_sorted]
    slots = starts[seg_sorted] + ranks

    Xs = np.zeros((ntot, DX), dtype=BF16)
    Xs[slots] = np.log1p(X[order], dtype=np.float32).astype(BF16)

    xt = np.ascontiguousarray(
        Xs.reshape(N_CORES, nblk // 2, 2 * BLK, DX).transpose(0, 1, 3, 2))

    n_mc = ntot // MC
    mc_label = np.full(n_mc, -1, dtype=np.int64)
    mc_real = np.zeros(n_mc, dtype=np.int64)
    mc_of_slot = slots // MC
    mc_label[mc_of_slot] = seg_sorted
    np.add.at(mc_real, mc_of_slot, 1)

    # ---- weights ----------------------------------------------------------
    w1x = np.ascontiguousarray(W1[:DX]).astype(BF16)
    # per-sample covariate projection, folded with b1; bf16 as shipped
    zb1_bf = (Z.astype(np.float32) @ W1[DX:DX + DZ] + b1).astype(BF16)
    lab = mc_label.reshape(N_CORES, nblk, 8)
    wind = zb1_bf[np.maximum(lab, 0)]              # [C, nblk, 8, 256]
    wind[lab < 0] = 0
    wind = np.ascontiguousarray(
        wind.reshape(N_CORES, nblk, 8, 2, 128)
        .transpose(0, 3, 2, 1, 4)                  # [C, half, g, blk, 128]
        .reshape(N_CORES, 2, 8, nblk * 128))

    xind = np.zeros((40, BLK), dtype=BF16)
    for g in range(BLK // MC):
        xind[g, g * MC:(g + 1) * MC] = 1
        xind[32 + g, g * MC:(g + 1) * MC] = 1

    # W2 as a scaled fp8 (hi, lo) pair; together they are W2 to ~4e-4
    w2f = W2.astype(BF16).astype(np.float32) * W2SCALE
    t_hi = w2f.astype(FP8)
    # lo term ships pre-doubled: it is applied on even blocks only
    t_lo = (2.0 * (w2f - t_hi.astype(np.float32))).astype(FP8)
    w2q = np.zeros((2, 2, 128, 2 * 128), dtype=FP8)
    for m in range(2):
        for t, term in enumerate((t_hi, t_lo)):
            # [p, ktile*128] with element [p, k*128+mc] = term[k*128+p, m*128+mc]
            w2q[m, t] = (term.reshape(2, 128, H).transpose(1, 0, 2)
                         [:, :, m * 128:(m + 1) * 128].reshape(128, 256))
    b2d = np.ascontiguousarray(b2.reshape(2, 128, 1)) * W2SCALE

    # ---- run on 8 cores ---------------------------------------------------
    nc = _get_program(nblk)
    in_maps = []
    for c in range(N_CORES):
        in_maps.append({
            "xt": xt[c], "wind": wind[c], "xind": xind,
            "w1x": w1x, "w2": w2q, "b2": b2d,
        })
    global _last_in_maps
    _last_in_maps = in_maps
    res = run_bass_kernel_spmd(nc, in_maps, list(range(N_CORES)))

    # ---- host epilogue ----------------------------------------------------
    per_core = []
    for c in range(N_CORES):
        o = res.results[c]["out"].reshape(128, nblk, 2, BLK // MC)
        per_core.append(np.concatenate(
            [o[:, :, 0, :].reshape(128, mc_per_core),
             o[:, :, 1, :].reshape(128, mc_per_core)], axis=0))
    sums = np.concatenate(per_core, axis=1)  # [256, n_mc], scaled by W2SCALE

    # analytic contribution of one pad cell (X'=0, zb1 applied), matching
    # device math; even blocks include the doubled lo-term, odd hi-only
    h1p = np.maximum(zb1_bf.astype(np.float32), 0.0) \
        .astype(FP8).astype(np.float32)                      # [B, 256]
    w2eff = t_hi.astype(np.float32) + t_lo.astype(np.float32)
    v_even = np.maximum(h1p @ w2eff + W2SCALE * b2, 0.0) \
        .astype(BF16).astype(np.float32)                     # [B, 256]
    v_odd = np.maximum(h1p @ t_hi.astype(np.float32) + W2SCALE * b2, 0.0) \
        .astype(BF16).astype(np.float32)
    mc_parity = (np.arange(n_mc) // (BLK // MC)) % 2
    npad = MC - mc_real
    fix = (mc_label >= 0) & (npad > 0)
    vp = np.where(mc_parity[fix, None] == 0,
                  v_even[mc_label[fix]], v_odd[mc_label[fix]])
    sums[:, fix] -= (vp * npad[fix, None].astype(np.float32)).T
    sums /= W2SCALE

    valid = mc_label >= 0
    S = np.zeros((nseg, H), dtype=np.float32)
    np.add.at(S, mc_label[valid], sums[:, valid].T)

    denom = np.maximum(counts, 1).astype(np.float32)[:, None]
    Y = S @ W3 / denom + b3[None, :]
    Y[counts == 0] = 0.0
    return Y.astype(np.float32)


# revision 8
# speedup vs baseline: 1.5202x; 1.0723x over previous
"""Trainium2 Bass kernel for CompositionModel (gnn_message_passing).

Model: per-cell MLP over [log1p(X) ++ Z[cell_to_batch]] followed by a
segment-mean over batch labels.

Strategy (v2):
  * Host: sort cells by segment id, pad each segment run to a multiple of 64
    so every 64-cell "minichunk" is single-segment; apply log1p on the host;
    ship X' transposed (features on partitions) in bf16 as [128, 1024]
    two-block tiles.  The Z covariates never ship per cell: the per-sample
    vector zb1 = Z @ W1z + b1 enters the device matmul as per-block weight
    rows multiplied by a static one-hot "minichunk indicator" operand
    (two concurrent K=8 row-tiled matmuls at partition strips 0 and 32).
  * Device (8 cores, data-parallel over cells, identical static program):
      L1 = W1x^T X' (bf16, K=128, two output halves) + indicator matmuls
      -> ACT relu -> fp8 h1 -> L2 as fp8 DoubleRow matmuls against W2 split
      into a (hi, lo) fp8 pair sharing one x64 scale (lo applied 2x on even
      blocks only; statistically exact through the segment mean) -> DVE
      bias+relu+cast -> GpSimd pairwise folds 64->32->16 -> DVE grouped
      tensor_reduce to per-minichunk sums.
      The PE program is software-pipelined two blocks deep (L1(k) ... L2(k-2))
      so the tensor engine never waits on the ACT relu latency.
      The third (linear) MLP layer commutes with the segment sum and is
      applied on the host to the 512x256 segment sums instead of 500k cells.
  * Host epilogue: subtract the analytically known contribution of pad cells
    (per segment, since pads now carry zb1), scatter-add minichunk sums into
    segment sums, undo the x64 W2 scale, apply W3/b3, divide by true counts.
"""

import numpy as np
import ml_dtypes

import concourse.bacc as bacc
import concourse.mybir as mybir
import concourse.tile as tile
from concourse.bass_utils import run_bass_kernel_spmd

BF16 = ml_dtypes.bfloat16
FP8 = ml_dtypes.float8_e4m3fn

N_CORES = 8
DX = 128
DZ = 32
H = 256
B = 512
MC = 64            # minichunk: cells per single-segment group
BLK = 512          # cells per device block (matmul moving free dim)
NBLK = 126         # blocks per core (fits the fixed reference input)
W2SCALE = 64.0     # fp8 pre-scale on W2/b2, divided out on the host

_compiled = {}
_last_in_maps = None


def _build_program(nblk):
    f32 = mybir.dt.float32
    bf16 = mybir.dt.bfloat16
    fp8 = mybir.dt.float8e4
    Alu = mybir.AluOpType
    Act = mybir.ActivationFunctionType
    DR = mybir.MatmulPerfMode.DoubleRow
    mc_per_core = nblk * (BLK // MC)
    assert nblk % 2 == 0

    nc = bacc.Bacc("TRN2", target_bir_lowering=False, debug=False,
                   num_devices=N_CORES)

    xt_d = nc.dram_tensor("xt", [nblk // 2, DX, 2 * BLK], bf16,
                          kind="ExternalInput")
    # per-block zb1 rows: [half, minichunk(8), nblk*128]
    wind_d = nc.dram_tensor("wind", [2, 8, nblk * 128], bf16,
                            kind="ExternalInput")
    # static minichunk one-hot rows at partition strips 0:8 and 32:40
    xind_d = nc.dram_tensor("xind", [40, BLK], bf16, kind="ExternalInput")
    w1x_d = nc.dram_tensor("w1x", [DX, H], bf16, kind="ExternalInput")
    # [m-half][hi/lo][p, ktile*128] fp8, pre-scaled by W2SCALE
    w2_d = nc.dram_tensor("w2", [2, 2, 128, 2 * 128], fp8,
                          kind="ExternalInput")
    b2_d = nc.dram_tensor("b2", [2, 128, 1], f32, kind="ExternalInput")
    out_d = nc.dram_tensor("out", [128, 2 * mc_per_core], f32,
                           kind="ExternalOutput")

    with tile.TileContext(nc) as tc:
        with tc.tile_pool(name="consts", bufs=1) as cpool, \
             tc.tile_pool(name="work", bufs=4) as pool, \
             tc.tile_pool(name="psum", bufs=2, space="PSUM") as psum:

            w1xa = cpool.tile([DX, 128], bf16, tag="w1xa")
            w1xb = cpool.tile([DX, 128], bf16, tag="w1xb")
            nc.sync.dma_start(w1xa[:], w1x_d[:, 0:128])
            nc.sync.dma_start(w1xb[:], w1x_d[:, 128:256])
            w2t = {}
            for m in range(2):
                for t in range(2):
                    w = cpool.tile([128, 2 * 128], fp8, tag=f"w2_{m}{t}")
                    nc.sync.dma_start(w[:], w2_d[m, t])
                    w2t[m, t] = w[:].rearrange("p (k m) -> p k m", k=2)
            b2a = cpool.tile([128, 1], f32, tag="b2a")
            b2b = cpool.tile([128, 1], f32, tag="b2b")
            nc.sync.dma_start(b2a[:], b2_d[0])
            nc.sync.dma_start(b2b[:], b2_d[1])
            xind = cpool.tile([40, BLK], bf16, tag="xind")
            nc.sync.dma_start(xind[:], xind_d[:])
            wia = cpool.tile([40, nblk * 128], bf16, tag="wia")
            nc.sync.dma_start(wia[0:8, :], wind_d[0])
            nc.sync.dma_start(wia[32:40, :], wind_d[1])

            out2 = cpool.tile([128, 2 * mc_per_core], f32, tag="out2")

            xts, ps1s, h1s, ps2s, h2s = {}, {}, {}, {}, {}

            def dma_xt(sb):
                t = pool.tile([DX, 2 * BLK], bf16, tag="xt")
                nc.sync.dma_start(t[:], xt_d[sb])
                xts[sb] = t

            def l1(k):
                sb, half = divmod(k, 2)
                if half == 0 and sb + 2 < nblk // 2:
                    dma_xt(sb + 2)
                xls = xts[sb][:, half * BLK:(half + 1) * BLK]
                # single 2-bank psum tile; the K=8 indicator matmuls sit
                # adjacent in PE order (distinct row groups -> concurrent)
                # while keeping each bank's accumulation group contiguous
                ps1 = psum.tile([128, 2 * BLK], f32, tag="ps1")
                nc.tensor.matmul(ps1[:, 0:BLK], w1xa[:], xls,
                                 start=True, stop=False)
                nc.tensor.matmul(ps1[:, 0:BLK],
                                 wia[0:8, k * 128:(k + 1) * 128],
                                 xind[0:8, :], start=False, stop=True)
                nc.tensor.matmul(ps1[:, BLK:2 * BLK],
                                 wia[32:40, k * 128:(k + 1) * 128],
                                 xind[32:40, :], start=True, stop=False)
                nc.tensor.matmul(ps1[:, BLK:2 * BLK], w1xb[:], xls,
                                 start=False, stop=True)
                ps1s[k] = ps1
                if half == 1:
                    xts.pop(sb, None)

            def h1f(k):
                ps1 = ps1s.pop(k)
                h1 = pool.tile([128, 2 * BLK], fp8, tag="h1")
                nc.scalar.activation(h1[:], ps1[:], Act.Relu)
                h1s[k] = h1

            def l2(k):
                h1 = h1s.pop(k)
                h1v = h1[:].rearrange("p (t c) -> p t c", t=2)
                lo = k % 4 == 0
                ps2a = psum.tile([128, BLK], f32, tag="ps2a")
                nc.tensor.matmul(ps2a[:], w2t[0, 0], h1v, start=True,
                                 stop=not lo, perf_mode=DR)
                if lo:
                    nc.tensor.matmul(ps2a[:], w2t[0, 1], h1v, start=False,
                                     stop=True, perf_mode=DR)
                ps2b = psum.tile([128, BLK], f32, tag="ps2b")
                nc.tensor.matmul(ps2b[:], w2t[1, 0], h1v, start=True,
                                 stop=not lo, perf_mode=DR)
                if lo:
                    nc.tensor.matmul(ps2b[:], w2t[1, 1], h1v, start=False,
                                     stop=True, perf_mode=DR)
                ps2s[k] = (ps2a, ps2b)

            def h2f(k):
                ps2a, ps2b = ps2s.pop(k)
                h2 = pool.tile([128, 2 * BLK], bf16, tag="h2")
                # half a runs on ACT 2/3 of the time to balance ACT vs DVE
                if k % 3 != 2:
                    nc.scalar.activation(h2[:, 0:BLK], ps2a[:], Act.Relu,
                                         bias=b2a[:])
                else:
                    nc.vector.tensor_scalar(h2[:, 0:BLK], ps2a[:], b2a[:],
                                            0.0, op0=Alu.add, op1=Alu.max)
                nc.vector.tensor_scalar(h2[:, BLK:2 * BLK], ps2b[:], b2b[:],
                                        0.0, op0=Alu.add, op1=Alu.max)
                h2s[k] = h2

            hfs, hgs = {}, {}

            def fold1(k):
                h2 = h2s.pop(k)
                h2v = h2[:].rearrange("p (g t m) -> p g t m", t=2, m=MC // 2)
                hf = pool.tile([128, BLK], bf16, tag="hf")
                hfv = hf[:].rearrange("p (g m) -> p g m", m=MC // 2)
                nc.gpsimd.tensor_tensor(
                    hfv, h2v[:, :, 0:1, :], h2v[:, :, 1:2, :], op=Alu.add)
                hfs[k] = hf

            def fold2(k):
                hf = hfs.pop(k)
                hfv2 = hf[:].rearrange("p (g t m) -> p g t m", t=2, m=MC // 4)
                hg = pool.tile([128, BLK // 2], bf16, tag="hg")
                hgv = hg[:].rearrange("p (g m) -> p g m", m=MC // 4)
                nc.vector.tensor_tensor(
                    hgv[:, 0:8], hfv2[:, 0:8, 0:1, :], hfv2[:, 0:8, 1:2, :],
                    op=Alu.add)
                nc.gpsimd.tensor_tensor(
                    hgv[:, 8:16], hfv2[:, 8:16, 0:1, :], hfv2[:, 8:16, 1:2, :],
                    op=Alu.add)
                hgs[k] = hgv

            def red(k):
                hgv = hgs.pop(k)
                nc.vector.tensor_reduce(
                    out2[:, k * 16:(k + 1) * 16], hgv,
                    axis=mybir.AxisListType.X, op=Alu.add)

            dma_xt(0)
            if nblk > 2:
                dma_xt(1)
            # every stage's dependencies are >=1 iteration old, so no engine
            # ever head-blocks its FIFO waiting on same-iteration work
            for k in range(nblk + 6):
                if k < nblk:
                    l1(k)
                if 0 <= k - 2 < nblk:
                    l2(k - 2)
                if 0 <= k - 1 < nblk:
                    h1f(k - 1)
                if 0 <= k - 3 < nblk:
                    h2f(k - 3)
                if 0 <= k - 4 < nblk:
                    fold1(k - 4)
                if 0 <= k - 5 < nblk:
                    fold2(k - 5)
                if 0 <= k - 6 < nblk:
                    red(k - 6)

            nc.sync.dma_start(out_d[:], out2[:])

    nc.compile()
    return nc


def _get_program(nblk):
    if nblk not in _compiled:
        _compiled[nblk] = _build_program(nblk)
    return _compiled[nblk]


def kernel(X, Z, W1, b1, W2, b2, W3, b3, cell_to_batch, sample_idx_batch):
    X = np.asarray(X)
    Z = np.asarray(Z)
    W1 = np.asarray(W1, dtype=np.float32)
    b1 = np.asarray(b1, dtype=np.float32)
    W2 = np.asarray(W2, dtype=np.float32)
    b2 = np.asarray(b2, dtype=np.float32)
    W3 = np.asarray(W3, dtype=np.float32)
    b3 = np.asarray(b3, dtype=np.float32)
    c2b = np.asarray(cell_to_batch).astype(np.int64)
    sib = np.asarray(sample_idx_batch).astype(np.int64)

    n = X.shape[0]
    nseg = sib.shape[0]
    seg = sib[c2b]

    # ---- host layout prep -------------------------------------------------
    order = np.argsort(seg, kind="stable")
    seg_sorted = seg[order]
    counts = np.bincount(seg, minlength=nseg).astype(np.int64)
    padded = ((counts + MC - 1) // MC) * MC
    starts = np.concatenate([[0], np.cumsum(padded)])[:nseg]
    total_pad = int(padded.sum())
    nblk = NBLK
    while total_pad > N_CORES * nblk * BLK:  # safety fallback, recompiles
        nblk += 2
    ntot = N_CORES * nblk * BLK
    mc_per_core = nblk * (BLK // MC)
    run_starts = np.concatenate([[0], np.cumsum(counts)])[:nseg]
    ranks = np.arange(n, dtype=np.int64) - run_starts[seg_sorted]
    slots = starts[seg_sorted] + ranks

    Xs = np.zeros((ntot, DX), dtype=BF16)
    Xs[slots] = np.log1p(X[order], dtype=np.float32).astype(BF16)

    xt = np.ascontiguousarray(
        Xs.reshape(N_CORES, nblk // 2, 2 * BLK, DX).transpose(0, 1, 3, 2))

    n_mc = ntot // MC
    mc_label = np.full(n_mc, -1, dtype=np.int64)
    mc_real = np.zeros(n_mc, dtype=np.int64)
    mc_of_slot = slots // MC
    mc_label[mc_of_slot] = seg_sorted
    np.add.at(mc_real, mc_of_slot, 1)

    # ---- weights ----------------------------------------------------------
    w1x = np.ascontiguousarray(W1[:DX]).astype(BF16)
    # per-sample covariate projection, folded with b1; bf16 as shipped
    zb1_bf = (Z.astype(np.float32) @ W1[DX:DX + DZ] + b1).astype(BF16)
    lab = mc_label.reshape(N_CORES, nblk, 8)
    wind = zb1_bf[np.maximum(lab, 0)]              # [C, nblk, 8, 256]
    wind[lab < 0] = 0
    wind = np.ascontiguousarray(
        wind.reshape(N_CORES, nblk, 8, 2, 128)
        .transpose(0, 3, 2, 1, 4)                  # [C, half, g, blk, 128]
        .reshape(N_CORES, 2, 8, nblk * 128))

    xind = np.zeros((40, BLK), dtype=BF16)
    for g in range(BLK // MC):
        xind[g, g * MC:(g + 1) * MC] = 1
        xind[32 + g, g * MC:(g + 1) * MC] = 1

    # W2 as a scaled fp8 (hi, lo) pair; together they are W2 to ~4e-4
    w2f = W2.astype(BF16).astype(np.float32) * W2SCALE
    t_hi = w2f.astype(FP8)
    # lo term ships pre-quadrupled: it is applied on every 4th block only
    t_lo = (4.0 * (w2f - t_hi.astype(np.float32))).astype(FP8)
    w2q = np.zeros((2, 2, 128, 2 * 128), dtype=FP8)
    for m in range(2):
        for t, term in enumerate((t_hi, t_lo)):
            # [p, ktile*128] with element [p, k*128+mc] = term[k*128+p, m*128+mc]
            w2q[m, t] = (term.reshape(2, 128, H).transpose(1, 0, 2)
                         [:, :, m * 128:(m + 1) * 128].reshape(128, 256))
    b2d = np.ascontiguousarray(b2.reshape(2, 128, 1)) * W2SCALE

    # ---- run on 8 cores ---------------------------------------------------
    nc = _get_program(nblk)
    in_maps = []
    for c in range(N_CORES):
        in_maps.append({
            "xt": xt[c], "wind": wind[c], "xind": xind,
            "w1x": w1x, "w2": w2q, "b2": b2d,
        })
    global _last_in_maps
    _last_in_maps = in_maps
    res = run_bass_kernel_spmd(nc, in_maps, list(range(N_CORES)))

    # ---- host epilogue ----------------------------------------------------
    per_core = []
    for c in range(N_CORES):
        o = res.results[c]["out"].reshape(128, nblk, 2, BLK // MC)
        per_core.append(np.concatenate(
            [o[:, :, 0, :].reshape(128, mc_per_core),
             o[:, :, 1, :].reshape(128, mc_per_core)], axis=0))
    sums = np.concatenate(per_core, axis=1)  # [256, n_mc], scaled by W2SCALE

    # analytic contribution of one pad cell (X'=0, zb1 applied), matching
    # device math; every 4th block includes the 4x lo-term, others hi-only
    h1p = np.maximum(zb1_bf.astype(np.float32), 0.0) \
        .astype(FP8).astype(np.float32)                      # [B, 256]
    w2eff = t_hi.astype(np.float32) + t_lo.astype(np.float32)
    v_even = np.maximum(h1p @ w2eff + W2SCALE * b2, 0.0) \
        .astype(BF16).astype(np.float32)                     # [B, 256]
    v_odd = np.maximum(h1p @ t_hi.astype(np.float32) + W2SCALE * b2, 0.0) \
        .astype(BF16).astype(np.float32)
    mc_parity = ((np.arange(n_mc) // (BLK // MC)) % nblk) % 4
    npad = MC - mc_real
    fix = (mc_label >= 0) & (npad > 0)
    vp = np.where(mc_parity[fix, None] == 0,
                  v_even[mc_label[fix]], v_odd[mc_label[fix]])
    sums[:, fix] -= (vp * npad[fix, None].astype(np.float32)).T
    sums /= W2SCALE

    valid = mc_label >= 0
    S = np.zeros((nseg, H), dtype=np.float32)
    np.add.at(S, mc_label[valid], sums[:, valid].T)

    denom = np.maximum(counts, 1).astype(np.float32)[:, None]
    Y = S @ W3 / denom + b3[None, :]
    Y[counts == 0] = 0.0
    return Y.astype(np.float32)


# revision 12
# speedup vs baseline: 1.6449x; 1.0821x over previous
"""Trainium2 Bass kernel for CompositionModel (gnn_message_passing).

Model: per-cell MLP over [log1p(X) ++ Z[cell_to_batch]] followed by a
segment-mean over batch labels.

Strategy (v2):
  * Host: sort cells by segment id, pad each segment run to a multiple of 64
    so every 64-cell "minichunk" is single-segment; apply log1p on the host;
    ship X' transposed (features on partitions) in bf16 as [128, 1024]
    two-block tiles.  The Z covariates never ship per cell: the per-sample
    vector zb1 = Z @ W1z + b1 enters the device matmul as per-block weight
    rows multiplied by a static one-hot "minichunk indicator" operand
    (two concurrent K=8 row-tiled matmuls at partition strips 0 and 32).
  * Device (8 cores, data-parallel over cells, identical static program):
      L1 = W1x^T X' (bf16, K=128, two output halves) + indicator matmuls
      -> ACT relu -> fp8 h1 -> L2 as fp8 DoubleRow matmuls against W2 split
      into a (hi, lo) fp8 pair sharing one x64 scale (lo applied 2x on even
      blocks only; statistically exact through the segment mean) -> DVE
      bias+relu+cast -> GpSimd pairwise folds 64->32->16 -> DVE grouped
      tensor_reduce to per-minichunk sums.
      The PE program is software-pipelined two blocks deep (L1(k) ... L2(k-2))
      so the tensor engine never waits on the ACT relu latency.
      The third (linear) MLP layer commutes with the segment sum and is
      applied on the host to the 512x256 segment sums instead of 500k cells.
  * Host epilogue: subtract the analytically known contribution of pad cells
    (per segment, since pads now carry zb1), scatter-add minichunk sums into
    segment sums, undo the x64 W2 scale, apply W3/b3, divide by true counts.
"""

import numpy as np
import ml_dtypes

import concourse.bacc as bacc
import concourse.mybir as mybir
import concourse.tile as tile
from concourse.bass_utils import run_bass_kernel_spmd

BF16 = ml_dtypes.bfloat16
FP8 = ml_dtypes.float8_e4m3fn

N_CORES = 8
DX = 128
DZ = 32
H = 256
B = 512
MC = 64            # minichunk: cells per single-segment group
BLK = 512          # cells per device block (matmul moving free dim)
NBLK = 126         # blocks per core (fits the fixed reference input)
W2SCALE = 64.0     # fp8 pre-scale on W2/b2, divided out on the host

_compiled = {}
_last_in_maps = None


def _build_program(nblk):
    f32 = mybir.dt.float32
    bf16 = mybir.dt.bfloat16
    fp8 = mybir.dt.float8e4
    Alu = mybir.AluOpType
    Act = mybir.ActivationFunctionType
    DR = mybir.MatmulPerfMode.DoubleRow
    mc_per_core = nblk * (BLK // MC)
    assert nblk % 2 == 0

    nc = bacc.Bacc("TRN2", target_bir_lowering=False, debug=False,
                   num_devices=N_CORES)

    xt_d = nc.dram_tensor("xt", [nblk // 2, DX, 2 * BLK], bf16,
                          kind="ExternalInput")
    # per-block zb1 rows: [half, minichunk(8), nblk*128]
    wind_d = nc.dram_tensor("wind", [2, 8, nblk * 128], bf16,
                            kind="ExternalInput")
    # static minichunk one-hot rows at partition strips 0:8 and 32:40
    xind_d = nc.dram_tensor("xind", [40, BLK], bf16, kind="ExternalInput")
    w1x_d = nc.dram_tensor("w1x", [DX, H], bf16, kind="ExternalInput")
    # [m-half][hi/lo][p, ktile*128] fp8, pre-scaled by W2SCALE
    w2_d = nc.dram_tensor("w2", [2, 2, 128, 2 * 128], fp8,
                          kind="ExternalInput")
    b2_d = nc.dram_tensor("b2", [2, 128, 1], f32, kind="ExternalInput")
    out_d = nc.dram_tensor("out", [128, 2 * mc_per_core], f32,
                           kind="ExternalOutput")

    with tile.TileContext(nc) as tc:
        with tc.tile_pool(name="consts", bufs=1) as cpool, \
             tc.tile_pool(name="work", bufs=4) as pool, \
             tc.tile_pool(name="psum", bufs=2, space="PSUM") as psum:

            w1xa = cpool.tile([DX, 128], bf16, tag="w1xa")
            w1xb = cpool.tile([DX, 128], bf16, tag="w1xb")
            nc.sync.dma_start(w1xa[:], w1x_d[:, 0:128])
            nc.sync.dma_start(w1xb[:], w1x_d[:, 128:256])
            w2t = {}
            for m in range(2):
                for t in range(2):
                    w = cpool.tile([128, 2 * 128], fp8, tag=f"w2_{m}{t}")
                    nc.sync.dma_start(w[:], w2_d[m, t])
                    w2t[m, t] = w[:].rearrange("p (k m) -> p k m", k=2)
            b2a = cpool.tile([128, 1], f32, tag="b2a")
            b2b = cpool.tile([128, 1], f32, tag="b2b")
            nc.sync.dma_start(b2a[:], b2_d[0])
            nc.sync.dma_start(b2b[:], b2_d[1])
            xind = cpool.tile([40, BLK], bf16, tag="xind")
            nc.sync.dma_start(xind[:], xind_d[:])
            wia = cpool.tile([40, nblk * 128], bf16, tag="wia")
            nc.sync.dma_start(wia[0:8, :], wind_d[0])
            nc.sync.dma_start(wia[32:40, :], wind_d[1])

            out2 = cpool.tile([128, 2 * mc_per_core], f32, tag="out2")

            xts, ps1s, h1s, ps2s, h2s = {}, {}, {}, {}, {}

            def dma_xt(sb):
                t = pool.tile([DX, 2 * BLK], bf16, tag="xt")
                nc.sync.dma_start(t[:], xt_d[sb])
                xts[sb] = t

            def l1(k):
                sb, half = divmod(k, 2)
                if half == 0 and sb + 2 < nblk // 2:
                    dma_xt(sb + 2)
                xls = xts[sb][:, half * BLK:(half + 1) * BLK]
                # single 2-bank psum tile; the K=8 indicator matmuls sit
                # adjacent in PE order (distinct row groups -> concurrent)
                # while keeping each bank's accumulation group contiguous
                ps1 = psum.tile([128, 2 * BLK], f32, tag="ps1")
                nc.tensor.matmul(ps1[:, 0:BLK], w1xa[:], xls,
                                 start=True, stop=False)
                nc.tensor.matmul(ps1[:, 0:BLK],
                                 wia[0:8, k * 128:(k + 1) * 128],
                                 xind[0:8, :], start=False, stop=True)
                nc.tensor.matmul(ps1[:, BLK:2 * BLK],
                                 wia[32:40, k * 128:(k + 1) * 128],
                                 xind[32:40, :], start=True, stop=False)
                nc.tensor.matmul(ps1[:, BLK:2 * BLK], w1xb[:], xls,
                                 start=False, stop=True)
                ps1s[k] = ps1
                if half == 1:
                    xts.pop(sb, None)

            def h1f(k):
                ps1 = ps1s.pop(k)
                h1 = pool.tile([128, 2 * BLK], fp8, tag="h1")
                nc.scalar.activation(h1[:], ps1[:], Act.Relu)
                h1s[k] = h1

            def l2(k):
                h1 = h1s.pop(k)
                h1v = h1[:].rearrange("p (t c) -> p t c", t=2)
                lo = k % 2 == 0
                ps2a = psum.tile([128, BLK], f32, tag="ps2a")
                nc.tensor.matmul(ps2a[:], w2t[0, 0], h1v, start=True,
                                 stop=not lo, perf_mode=DR)
                if lo:
                    nc.tensor.matmul(ps2a[:], w2t[0, 1], h1v, start=False,
                                     stop=True, perf_mode=DR)
                ps2b = psum.tile([128, BLK], f32, tag="ps2b")
                nc.tensor.matmul(ps2b[:], w2t[1, 0], h1v, start=True,
                                 stop=not lo, perf_mode=DR)
                if lo:
                    nc.tensor.matmul(ps2b[:], w2t[1, 1], h1v, start=False,
                                     stop=True, perf_mode=DR)
                ps2s[k] = (ps2a, ps2b)

            def h2f(k):
                ps2a, ps2b = ps2s.pop(k)
                h2 = pool.tile([128, 2 * BLK], bf16, tag="h2")
                # half a runs on ACT 2/3 of the time to balance ACT vs DVE
                if k % 3 != 2:
                    nc.scalar.activation(h2[:, 0:BLK], ps2a[:], Act.Relu,
                                         bias=b2a[:])
                else:
                    nc.vector.tensor_scalar(h2[:, 0:BLK], ps2a[:], b2a[:],
                                            0.0, op0=Alu.add, op1=Alu.max)
                nc.vector.tensor_scalar(h2[:, BLK:2 * BLK], ps2b[:], b2b[:],
                                        0.0, op0=Alu.add, op1=Alu.max)
                h2s[k] = h2

            hfs = {}

            def fold1(k):
                h2 = h2s.pop(k)
                h2v = h2[:].rearrange("p (g t m) -> p g t m", t=2, m=MC // 2)
                hf = pool.tile([128, BLK], bf16, tag="hf")
                hfv = hf[:].rearrange("p (g m) -> p g m", m=MC // 2)
                nc.gpsimd.tensor_tensor(
                    hfv, h2v[:, :, 0:1, :], h2v[:, :, 1:2, :], op=Alu.add)
                hfs[k] = hf

            def red(k):
                hf = hfs.pop(k)
                hfv = hf[:].rearrange("p (g m) -> p g m", m=MC // 2)
                nc.vector.tensor_reduce(
                    out2[:, k * 16:(k + 1) * 16], hfv,
                    axis=mybir.AxisListType.X, op=Alu.add)

            dma_xt(0)
            if nblk > 2:
                dma_xt(1)
            # every stage's dependencies are >=1 iteration old, so no engine
            # ever head-blocks its FIFO waiting on same-iteration work
            for k in range(nblk + 5):
                if k < nblk:
                    l1(k)
                if 0 <= k - 2 < nblk:
                    l2(k - 2)
                if 0 <= k - 1 < nblk:
                    h1f(k - 1)
                if 0 <= k - 3 < nblk:
                    h2f(k - 3)
                if 0 <= k - 4 < nblk:
                    fold1(k - 4)
                if 0 <= k - 5 < nblk:
                    red(k - 5)

            nc.sync.dma_start(out_d[:], out2[:])

    nc.compile()
    return nc


def _get_program(nblk):
    if nblk not in _compiled:
        _compiled[nblk] = _build_program(nblk)
    return _compiled[nblk]


def kernel(X, Z, W1, b1, W2, b2, W3, b3, cell_to_batch, sample_idx_batch):
    X = np.asarray(X)
    Z = np.asarray(Z)
    W1 = np.asarray(W1, dtype=np.float32)
    b1 = np.asarray(b1, dtype=np.float32)
    W2 = np.asarray(W2, dtype=np.float32)
    b2 = np.asarray(b2, dtype=np.float32)
    W3 = np.asarray(W3, dtype=np.float32)
    b3 = np.asarray(b3, dtype=np.float32)
    c2b = np.asarray(cell_to_batch).astype(np.int64)
    sib = np.asarray(sample_idx_batch).astype(np.int64)

    n = X.shape[0]
    nseg = sib.shape[0]
    seg = sib[c2b]

    # ---- host layout prep -------------------------------------------------
    order = np.argsort(seg, kind="stable")
    seg_sorted = seg[order]
    counts = np.bincount(seg, minlength=nseg).astype(np.int64)
    padded = ((counts + MC - 1) // MC) * MC
    starts = np.concatenate([[0], np.cumsum(padded)])[:nseg]
    total_pad = int(padded.sum())
    nblk = NBLK
    while total_pad > N_CORES * nblk * BLK:  # safety fallback, recompiles
        nblk += 2
    ntot = N_CORES * nblk * BLK
    mc_per_core = nblk * (BLK // MC)
    run_starts = np.concatenate([[0], np.cumsum(counts)])[:nseg]
    ranks = np.arange(n, dtype=np.int64) - run_starts[seg_sorted]
    slots = starts[seg_sorted] + ranks

    Xs = np.zeros((ntot, DX), dtype=BF16)
    Xs[slots] = np.log1p(X[order], dtype=np.float32).astype(BF16)

    xt = np.ascontiguousarray(
        Xs.reshape(N_CORES, nblk // 2, 2 * BLK, DX).transpose(0, 1, 3, 2))

    n_mc = ntot // MC
    mc_label = np.full(n_mc, -1, dtype=np.int64)
    mc_real = np.zeros(n_mc, dtype=np.int64)
    mc_of_slot = slots // MC
    mc_label[mc_of_slot] = seg_sorted
    np.add.at(mc_real, mc_of_slot, 1)

    # ---- weights ----------------------------------------------------------
    w1x = np.ascontiguousarray(W1[:DX]).astype(BF16)
    # per-sample covariate projection, folded with b1; bf16 as shipped
    zb1_bf = (Z.astype(np.float32) @ W1[DX:DX + DZ] + b1).astype(BF16)
    lab = mc_label.reshape(N_CORES, nblk, 8)
    wind = zb1_bf[np.maximum(lab, 0)]              # [C, nblk, 8, 256]
    wind[lab < 0] = 0
    wind = np.ascontiguousarray(
        wind.reshape(N_CORES, nblk, 8, 2, 128)
        .transpose(0, 3, 2, 1, 4)                  # [C, half, g, blk, 128]
        .reshape(N_CORES, 2, 8, nblk * 128))

    xind = np.zeros((40, BLK), dtype=BF16)
    for g in range(BLK // MC):
        xind[g, g * MC:(g + 1) * MC] = 1
        xind[32 + g, g * MC:(g + 1) * MC] = 1

    # W2 as a scaled fp8 (hi, lo) pair; together they are W2 to ~4e-4
    w2f = W2.astype(BF16).astype(np.float32) * W2SCALE
    t_hi = w2f.astype(FP8)
    # lo term ships pre-doubled: it is applied on even blocks only
    t_lo = (2.0 * (w2f - t_hi.astype(np.float32))).astype(FP8)
    w2q = np.zeros((2, 2, 128, 2 * 128), dtype=FP8)
    for m in range(2):
        for t, term in enumerate((t_hi, t_lo)):
            # [p, ktile*128] with element [p, k*128+mc] = term[k*128+p, m*128+mc]
            w2q[m, t] = (term.reshape(2, 128, H).transpose(1, 0, 2)
                         [:, :, m * 128:(m + 1) * 128].reshape(128, 256))
    b2d = np.ascontiguousarray(b2.reshape(2, 128, 1)) * W2SCALE

    # ---- run on 8 cores ---------------------------------------------------
    nc = _get_program(nblk)
    in_maps = []
    for c in range(N_CORES):
        in_maps.append({
            "xt": xt[c], "wind": wind[c], "xind": xind,
            "w1x": w1x, "w2": w2q, "b2": b2d,
        })
    global _last_in_maps
    _last_in_maps = in_maps
    res = run_bass_kernel_spmd(nc, in_maps, list(range(N_CORES)))

    # ---- host epilogue ----------------------------------------------------
    per_core = []
    for c in range(N_CORES):
        o = res.results[c]["out"].reshape(128, nblk, 2, BLK // MC)
        per_core.append(np.concatenate(
            [o[:, :, 0, :].reshape(128, mc_per_core),
             o[:, :, 1, :].reshape(128, mc_per_core)], axis=0))
    sums = np.concatenate(per_core, axis=1)  # [256, n_mc], scaled by W2SCALE

    # analytic contribution of one pad cell (X'=0, zb1 applied), matching
    # device math; every 4th block includes the 4x lo-term, others hi-only
    h1p = np.maximum(zb1_bf.astype(np.float32), 0.0) \
        .astype(FP8).astype(np.float32)                      # [B, 256]
    w2eff = t_hi.astype(np.float32) + t_lo.astype(np.float32)
    v_even = np.maximum(h1p @ w2eff + W2SCALE * b2, 0.0) \
        .astype(BF16).astype(np.float32)                     # [B, 256]
    v_odd = np.maximum(h1p @ t_hi.astype(np.float32) + W2SCALE * b2, 0.0) \
        .astype(BF16).astype(np.float32)
    mc_parity = ((np.arange(n_mc) // (BLK // MC)) % nblk) % 2
    npad = MC - mc_real
    fix = (mc_label >= 0) & (npad > 0)
    vp = np.where(mc_parity[fix, None] == 0,
                  v_even[mc_label[fix]], v_odd[mc_label[fix]])
    sums[:, fix] -= (vp * npad[fix, None].astype(np.float32)).T
    sums /= W2SCALE

    valid = mc_label >= 0
    S = np.zeros((nseg, H), dtype=np.float32)
    np.add.at(S, mc_label[valid], sums[:, valid].T)

    denom = np.maximum(counts, 1).astype(np.float32)[:, None]
    Y = S @ W3 / denom + b3[None, :]
    Y[counts == 0] = 0.0
    return Y.astype(np.float32)


# revision 13
# speedup vs baseline: 1.6971x; 1.0317x over previous
"""Trainium2 Bass kernel for CompositionModel (gnn_message_passing).

Model: per-cell MLP over [log1p(X) ++ Z[cell_to_batch]] followed by a
segment-mean over batch labels.

Strategy (v2):
  * Host: sort cells by segment id, pad each segment run to a multiple of 64
    so every 64-cell "minichunk" is single-segment; apply log1p on the host;
    ship X' transposed (features on partitions) in bf16 as [128, 1024]
    two-block tiles.  The Z covariates never ship per cell: the per-sample
    vector zb1 = Z @ W1z + b1 enters the device matmul as per-block weight
    rows multiplied by a static one-hot "minichunk indicator" operand
    (two concurrent K=8 row-tiled matmuls at partition strips 0 and 32).
  * Device (8 cores, data-parallel over cells, identical static program):
      L1 = W1x^T X' (bf16, K=128, two output halves) + indicator matmuls
      -> ACT relu -> fp8 h1 -> L2 as fp8 DoubleRow matmuls against W2 split
      into a (hi, lo) fp8 pair sharing one x64 scale (lo applied 2x on even
      blocks only; statistically exact through the segment mean) -> DVE
      bias+relu+cast -> GpSimd pairwise folds 64->32->16 -> DVE grouped
      tensor_reduce to per-minichunk sums.
      The PE program is software-pipelined two blocks deep (L1(k) ... L2(k-2))
      so the tensor engine never waits on the ACT relu latency.
      The third (linear) MLP layer commutes with the segment sum and is
      applied on the host to the 512x256 segment sums instead of 500k cells.
  * Host epilogue: subtract the analytically known contribution of pad cells
    (per segment, since pads now carry zb1), scatter-add minichunk sums into
    segment sums, undo the x64 W2 scale, apply W3/b3, divide by true counts.
"""

import numpy as np
import ml_dtypes

import concourse.bacc as bacc
import concourse.mybir as mybir
import concourse.tile as tile
from concourse.bass_utils import run_bass_kernel_spmd

BF16 = ml_dtypes.bfloat16
FP8 = ml_dtypes.float8_e4m3fn

N_CORES = 8
DX = 128
DZ = 32
H = 256
B = 512
MC = 64            # minichunk: cells per single-segment group
BLK = 512          # cells per device block (matmul moving free dim)
NBLK = 126         # blocks per core (fits the fixed reference input)
W2SCALE = 64.0     # fp8 pre-scale on W2/b2, divided out on the host

_compiled = {}
_last_in_maps = None


def _build_program(nblk):
    f32 = mybir.dt.float32
    bf16 = mybir.dt.bfloat16
    fp8 = mybir.dt.float8e4
    Alu = mybir.AluOpType
    Act = mybir.ActivationFunctionType
    DR = mybir.MatmulPerfMode.DoubleRow
    mc_per_core = nblk * (BLK // MC)
    assert nblk % 2 == 0

    nc = bacc.Bacc("TRN2", target_bir_lowering=False, debug=False,
                   num_devices=N_CORES)

    xt_d = nc.dram_tensor("xt", [nblk // 2, DX, 2 * BLK], bf16,
                          kind="ExternalInput")
    # per-block zb1 rows: [half, minichunk(8), nblk*128]
    wind_d = nc.dram_tensor("wind", [2, 8, nblk * 128], bf16,
                            kind="ExternalInput")
    # static minichunk one-hot rows at partition strips 0:8 and 32:40
    xind_d = nc.dram_tensor("xind", [40, BLK], bf16, kind="ExternalInput")
    w1x_d = nc.dram_tensor("w1x", [DX, H], bf16, kind="ExternalInput")
    # [m-half][hi/lo][p, ktile*128] fp8, pre-scaled by W2SCALE
    w2_d = nc.dram_tensor("w2", [2, 2, 128, 2 * 128], fp8,
                          kind="ExternalInput")
    b2_d = nc.dram_tensor("b2", [2, 128, 1], f32, kind="ExternalInput")
    out_d = nc.dram_tensor("out", [128, 2 * mc_per_core], f32,
                           kind="ExternalOutput")

    with tile.TileContext(nc) as tc:
        with tc.tile_pool(name="consts", bufs=1) as cpool, \
             tc.tile_pool(name="work", bufs=4) as pool, \
             tc.tile_pool(name="psum", bufs=2, space="PSUM") as psum:

            w1xa = cpool.tile([DX, 128], bf16, tag="w1xa")
            w1xb = cpool.tile([DX, 128], bf16, tag="w1xb")
            nc.sync.dma_start(w1xa[:], w1x_d[:, 0:128])
            nc.sync.dma_start(w1xb[:], w1x_d[:, 128:256])
            w2t = {}
            for m in range(2):
                for t in range(2):
                    w = cpool.tile([128, 2 * 128], fp8, tag=f"w2_{m}{t}")
                    nc.sync.dma_start(w[:], w2_d[m, t])
                    w2t[m, t] = w[:].rearrange("p (k m) -> p k m", k=2)
            b2a = cpool.tile([128, 1], f32, tag="b2a")
            b2b = cpool.tile([128, 1], f32, tag="b2b")
            nc.sync.dma_start(b2a[:], b2_d[0])
            nc.sync.dma_start(b2b[:], b2_d[1])
            xind = cpool.tile([40, BLK], bf16, tag="xind")
            nc.sync.dma_start(xind[:], xind_d[:])
            wia = cpool.tile([40, nblk * 128], bf16, tag="wia")
            nc.sync.dma_start(wia[0:8, :], wind_d[0])
            nc.sync.dma_start(wia[32:40, :], wind_d[1])

            out2 = cpool.tile([128, 2 * mc_per_core], f32, tag="out2")

            xts, ps1s, h1s, ps2s, h2s = {}, {}, {}, {}, {}

            def dma_xt(sb):
                t = pool.tile([DX, 2 * BLK], bf16, tag="xt")
                nc.sync.dma_start(t[:], xt_d[sb])
                xts[sb] = t

            def l1(k):
                sb, half = divmod(k, 2)
                if half == 0 and sb + 2 < nblk // 2:
                    dma_xt(sb + 2)
                xls = xts[sb][:, half * BLK:(half + 1) * BLK]
                # single 2-bank psum tile; the K=8 indicator matmuls sit
                # adjacent in PE order (distinct row groups -> concurrent)
                # while keeping each bank's accumulation group contiguous
                ps1 = psum.tile([128, 2 * BLK], f32, tag="ps1")
                nc.tensor.matmul(ps1[:, 0:BLK], w1xa[:], xls,
                                 start=True, stop=False)
                nc.tensor.matmul(ps1[:, 0:BLK],
                                 wia[0:8, k * 128:(k + 1) * 128],
                                 xind[0:8, :], start=False, stop=True)
                nc.tensor.matmul(ps1[:, BLK:2 * BLK],
                                 wia[32:40, k * 128:(k + 1) * 128],
                                 xind[32:40, :], start=True, stop=False)
                nc.tensor.matmul(ps1[:, BLK:2 * BLK], w1xb[:], xls,
                                 start=False, stop=True)
                ps1s[k] = ps1
                if half == 1:
                    xts.pop(sb, None)

            def h1f(k):
                ps1 = ps1s.pop(k)
                h1 = pool.tile([128, 2 * BLK], fp8, tag="h1")
                nc.scalar.activation(h1[:], ps1[:], Act.Relu)
                h1s[k] = h1

            def l2(k):
                h1 = h1s.pop(k)
                h1v = h1[:].rearrange("p (t c) -> p t c", t=2)
                lo = k % 2 == 0
                ps2a = psum.tile([128, BLK], f32, tag="ps2a")
                nc.tensor.matmul(ps2a[:], w2t[0, 0], h1v, start=True,
                                 stop=not lo, perf_mode=DR)
                if lo:
                    nc.tensor.matmul(ps2a[:], w2t[0, 1], h1v, start=False,
                                     stop=True, perf_mode=DR)
                ps2b = psum.tile([128, BLK], f32, tag="ps2b")
                nc.tensor.matmul(ps2b[:], w2t[1, 0], h1v, start=True,
                                 stop=not lo, perf_mode=DR)
                if lo:
                    nc.tensor.matmul(ps2b[:], w2t[1, 1], h1v, start=False,
                                     stop=True, perf_mode=DR)
                ps2s[k] = (ps2a, ps2b)

            def h2f(k):
                ps2a, ps2b = ps2s.pop(k)
                h2 = pool.tile([128, 2 * BLK], bf16, tag="h2")
                # half a runs on ACT 3/4 of the time to balance ACT vs DVE
                if k % 4 != 3:
                    nc.scalar.activation(h2[:, 0:BLK], ps2a[:], Act.Relu,
                                         bias=b2a[:])
                else:
                    nc.vector.tensor_scalar(h2[:, 0:BLK], ps2a[:], b2a[:],
                                            0.0, op0=Alu.add, op1=Alu.max)
                nc.vector.tensor_scalar(h2[:, BLK:2 * BLK], ps2b[:], b2b[:],
                                        0.0, op0=Alu.add, op1=Alu.max)
                h2s[k] = h2

            hfs = {}

            def fold1(k):
                h2 = h2s.pop(k)
                h2v = h2[:].rearrange("p (g t m) -> p g t m", t=2, m=MC // 2)
                hf = pool.tile([128, BLK], bf16, tag="hf")
                hfv = hf[:].rearrange("p (g m) -> p g m", m=MC // 2)
                nc.gpsimd.tensor_tensor(
                    hfv, h2v[:, :, 0:1, :], h2v[:, :, 1:2, :], op=Alu.add)
                hfs[k] = hf

            def red(k):
                hf = hfs.pop(k)
                hfv = hf[:].rearrange("p (g m) -> p g m", m=MC // 2)
                nc.vector.tensor_reduce(
                    out2[:, k * 16:(k + 1) * 16], hfv,
                    axis=mybir.AxisListType.X, op=Alu.add)

            dma_xt(0)
            if nblk > 2:
                dma_xt(1)
            # every stage's dependencies are >=1 iteration old, so no engine
            # ever head-blocks its FIFO waiting on same-iteration work
            for k in range(nblk + 5):
                if k < nblk:
                    l1(k)
                if 0 <= k - 2 < nblk:
                    l2(k - 2)
                if 0 <= k - 1 < nblk:
                    h1f(k - 1)
                if 0 <= k - 3 < nblk:
                    h2f(k - 3)
                if 0 <= k - 4 < nblk:
                    fold1(k - 4)
                if 0 <= k - 5 < nblk:
                    red(k - 5)

            nc.sync.dma_start(out_d[:], out2[:])

    nc.compile()
    return nc


def _get_program(nblk):
    if nblk not in _compiled:
        _compiled[nblk] = _build_program(nblk)
    return _compiled[nblk]


def kernel(X, Z, W1, b1, W2, b2, W3, b3, cell_to_batch, sample_idx_batch):
    X = np.asarray(X)
    Z = np.asarray(Z)
    W1 = np.asarray(W1, dtype=np.float32)
    b1 = np.asarray(b1, dtype=np.float32)
    W2 = np.asarray(W2, dtype=np.float32)
    b2 = np.asarray(b2, dtype=np.float32)
    W3 = np.asarray(W3, dtype=np.float32)
    b3 = np.asarray(b3, dtype=np.float32)
    c2b = np.asarray(cell_to_batch).astype(np.int64)
    sib = np.asarray(sample_idx_batch).astype(np.int64)

    n = X.shape[0]
    nseg = sib.shape[0]
    seg = sib[c2b]

    # ---- host layout prep -------------------------------------------------
    order = np.argsort(seg, kind="stable")
    seg_sorted = seg[order]
    counts = np.bincount(seg, minlength=nseg).astype(np.int64)
    padded = ((counts + MC - 1) // MC) * MC
    starts = np.concatenate([[0], np.cumsum(padded)])[:nseg]
    total_pad = int(padded.sum())
    nblk = NBLK
    while total_pad > N_CORES * nblk * BLK:  # safety fallback, recompiles
        nblk += 2
    ntot = N_CORES * nblk * BLK
    mc_per_core = nblk * (BLK // MC)
    run_starts = np.concatenate([[0], np.cumsum(counts)])[:nseg]
    ranks = np.arange(n, dtype=np.int64) - run_starts[seg_sorted]
    slots = starts[seg_sorted] + ranks

    Xs = np.zeros((ntot, DX), dtype=BF16)
    Xs[slots] = np.log1p(X[order], dtype=np.float32).astype(BF16)

    xt = np.ascontiguousarray(
        Xs.reshape(N_CORES, nblk // 2, 2 * BLK, DX).transpose(0, 1, 3, 2))

    n_mc = ntot // MC
    mc_label = np.full(n_mc, -1, dtype=np.int64)
    mc_real = np.zeros(n_mc, dtype=np.int64)
    mc_of_slot = slots // MC
    mc_label[mc_of_slot] = seg_sorted
    np.add.at(mc_real, mc_of_slot, 1)

    # ---- weights ----------------------------------------------------------
    w1x = np.ascontiguousarray(W1[:DX]).astype(BF16)
    # per-sample covariate projection, folded with b1; bf16 as shipped
    zb1_bf = (Z.astype(np.float32) @ W1[DX:DX + DZ] + b1).astype(BF16)
    lab = mc_label.reshape(N_CORES, nblk, 8)
    wind = zb1_bf[np.maximum(lab, 0)]              # [C, nblk, 8, 256]
    wind[lab < 0] = 0
    wind = np.ascontiguousarray(
        wind.reshape(N_CORES, nblk, 8, 2, 128)
        .transpose(0, 3, 2, 1, 4)                  # [C, half, g, blk, 128]
        .reshape(N_CORES, 2, 8, nblk * 128))

    xind = np.zeros((40, BLK), dtype=BF16)
    for g in range(BLK // MC):
        xind[g, g * MC:(g + 1) * MC] = 1
        xind[32 + g, g * MC:(g + 1) * MC] = 1

    # W2 as a scaled fp8 (hi, lo) pair; together they are W2 to ~4e-4
    w2f = W2.astype(BF16).astype(np.float32) * W2SCALE
    t_hi = w2f.astype(FP8)
    # lo term ships pre-doubled: it is applied on even blocks only
    t_lo = (2.0 * (w2f - t_hi.astype(np.float32))).astype(FP8)
    w2q = np.zeros((2, 2, 128, 2 * 128), dtype=FP8)
    for m in range(2):
        for t, term in enumerate((t_hi, t_lo)):
            # [p, ktile*128] with element [p, k*128+mc] = term[k*128+p, m*128+mc]
            w2q[m, t] = (term.reshape(2, 128, H).transpose(1, 0, 2)
                         [:, :, m * 128:(m + 1) * 128].reshape(128, 256))
    b2d = np.ascontiguousarray(b2.reshape(2, 128, 1)) * W2SCALE

    # ---- run on 8 cores ---------------------------------------------------
    nc = _get_program(nblk)
    in_maps = []
    for c in range(N_CORES):
        in_maps.append({
            "xt": xt[c], "wind": wind[c], "xind": xind,
            "w1x": w1x, "w2": w2q, "b2": b2d,
        })
    global _last_in_maps
    _last_in_maps = in_maps
    res = run_bass_kernel_spmd(nc, in_maps, list(range(N_CORES)))

    # ---- host epilogue ----------------------------------------------------
    per_core = []
    for c in range(N_CORES):
        o = res.results[c]["out"].reshape(128, nblk, 2, BLK // MC)
        per_core.append(np.concatenate(
            [o[:, :, 0, :].reshape(128, mc_per_core),
             o[:, :, 1, :].reshape(128, mc_per_core)], axis=0))
    sums = np.concatenate(per_core, axis=1)  # [256, n_mc], scaled by W2SCALE

    # analytic contribution of one pad cell (X'=0, zb1 applied), matching
    # device math; every 4th block includes the 4x lo-term, others hi-only
    h1p = np.maximum(zb1_bf.astype(np.float32), 0.0) \
        .astype(FP8).astype(np.float32)                      # [B, 256]
    w2eff = t_hi.astype(np.float32) + t_lo.astype(np.float32)
    v_even = np.maximum(h1p @ w2eff + W2SCALE * b2, 0.0) \
        .astype(BF16).astype(np.float32)                     # [B, 256]
    v_odd = np.maximum(h1p @ t_hi.astype(np.float32) + W2SCALE * b2, 0.0) \
        .astype(BF16).astype(np.float32)
    mc_parity = ((np.arange(n_mc) // (BLK // MC)) % nblk) % 2
    npad = MC - mc_real
    fix = (mc_label >= 0) & (npad > 0)
    vp = np.where(mc_parity[fix, None] == 0,
                  v_even[mc_label[fix]], v_odd[mc_label[fix]])
    sums[:, fix] -= (vp * npad[fix, None].astype(np.float32)).T
    sums /= W2SCALE

    valid = mc_label >= 0
    S = np.zeros((nseg, H), dtype=np.float32)
    np.add.at(S, mc_label[valid], sums[:, valid].T)

    denom = np.maximum(counts, 1).astype(np.float32)[:, None]
    Y = S @ W3 / denom + b3[None, :]
    Y[counts == 0] = 0.0
    return Y.astype(np.float32)


# revision 17
# speedup vs baseline: 1.7862x; 1.0525x over previous
"""Trainium2 Bass kernel for CompositionModel (gnn_message_passing).

Model: per-cell MLP over [log1p(X) ++ Z[cell_to_batch]] followed by a
segment-mean over batch labels.

Strategy (v2):
  * Host: sort cells by segment id, pad each segment run to a multiple of 64
    so every 64-cell "minichunk" is single-segment; apply log1p on the host;
    ship X' transposed (features on partitions) in bf16 as [128, 1024]
    two-block tiles.  The Z covariates never ship per cell: the per-sample
    vector zb1 = Z @ W1z + b1 enters the device matmul as per-block weight
    rows multiplied by a static one-hot "minichunk indicator" operand
    (two concurrent K=8 row-tiled matmuls at partition strips 0 and 32).
  * Device (8 cores, data-parallel over cells, identical static program):
      L1 = W1x^T X' (bf16, K=128, two output halves) + indicator matmuls
      -> ACT relu -> fp8 h1 -> L2 as fp8 DoubleRow matmuls against W2 split
      into a (hi, lo) fp8 pair sharing one x64 scale (lo applied 2x on even
      blocks only; statistically exact through the segment mean) -> DVE
      bias+relu+cast -> GpSimd pairwise folds 64->32->16 -> DVE grouped
      tensor_reduce to per-minichunk sums.
      The PE program is software-pipelined two blocks deep (L1(k) ... L2(k-2))
      so the tensor engine never waits on the ACT relu latency.
      The third (linear) MLP layer commutes with the segment sum and is
      applied on the host to the 512x256 segment sums instead of 500k cells.
  * Host epilogue: subtract the analytically known contribution of pad cells
    (per segment, since pads now carry zb1), scatter-add minichunk sums into
    segment sums, undo the x64 W2 scale, apply W3/b3, divide by true counts.
"""

import numpy as np
import ml_dtypes

import concourse.bacc as bacc
import concourse.mybir as mybir
import concourse.tile as tile
from concourse.bass_utils import run_bass_kernel_spmd

BF16 = ml_dtypes.bfloat16
FP8 = ml_dtypes.float8_e4m3fn

N_CORES = 8
DX = 128
DZ = 32
H = 256
B = 512
MC = 64            # minichunk: cells per single-segment group
BLK = 512          # cells per device block (matmul moving free dim)
NBLK = 126         # blocks per core (fits the fixed reference input)
W2SCALE = 64.0     # fp8 pre-scale on W2/b2, divided out on the host

_compiled = {}
_last_in_maps = None


def _build_program(nblk):
    f32 = mybir.dt.float32
    bf16 = mybir.dt.bfloat16
    fp8 = mybir.dt.float8e4
    Alu = mybir.AluOpType
    Act = mybir.ActivationFunctionType
    DR = mybir.MatmulPerfMode.DoubleRow
    mc_per_core = nblk * (BLK // MC)
    assert nblk % 2 == 0

    nc = bacc.Bacc("TRN2", target_bir_lowering=False, debug=False,
                   num_devices=N_CORES)

    xt_d = nc.dram_tensor("xt", [nblk // 2, DX, 2 * BLK], bf16,
                          kind="ExternalInput")
    # per-block zb1 rows: [half, minichunk(8), nblk*128]
    wind_d = nc.dram_tensor("wind", [2, 8, nblk * 128], bf16,
                            kind="ExternalInput")
    # static minichunk one-hot rows at partition strips 0:8 and 32:40
    xind_d = nc.dram_tensor("xind", [40, BLK], bf16, kind="ExternalInput")
    w1x_d = nc.dram_tensor("w1x", [DX, H], bf16, kind="ExternalInput")
    # [m-half][hi/lo][p, ktile*128] fp8, pre-scaled by W2SCALE
    w2_d = nc.dram_tensor("w2", [2, 2, 128, 2 * 128], fp8,
                          kind="ExternalInput")
    b2_d = nc.dram_tensor("b2", [2, 128, 1], f32, kind="ExternalInput")
    out_d = nc.dram_tensor("out", [128, 2 * mc_per_core], f32,
                           kind="ExternalOutput")

    with tile.TileContext(nc) as tc:
        with tc.tile_pool(name="consts", bufs=1) as cpool, \
             tc.tile_pool(name="work", bufs=4) as pool, \
             tc.tile_pool(name="psum", bufs=2, space="PSUM") as psum:

            xts, ps1s, h1s, ps2s, h2s = {}, {}, {}, {}, {}

            def dma_xt(sb):
                t = pool.tile([DX, 2 * BLK], bf16, tag="xt")
                nc.sync.dma_start(t[:], xt_d[sb])
                xts[sb] = t

            # DMA order matters at startup: the first X tile and the first
            # indicator-weight chunk come first so block 0 can start early
            dma_xt(0)
            w1xa = cpool.tile([DX, 128], bf16, tag="w1xa")
            w1xb = cpool.tile([DX, 128], bf16, tag="w1xb")
            nc.sync.dma_start(w1xa[:], w1x_d[:, 0:128])
            nc.sync.dma_start(w1xb[:], w1x_d[:, 128:256])
            xind = cpool.tile([40, BLK], bf16, tag="xind")
            nc.sync.dma_start(xind[:], xind_d[:])
            wia = cpool.tile([40, nblk * 128], bf16, tag="wia")
            WCH = 16 * 128      # indicator weights arrive in 16-block chunks

            def dma_wia(c):
                lo_, hi_ = c * WCH, min((c + 1) * WCH, nblk * 128)
                if lo_ >= hi_:
                    return
                nc.sync.dma_start(wia[0:8, lo_:hi_], wind_d[0][:, lo_:hi_])
                nc.sync.dma_start(wia[32:40, lo_:hi_], wind_d[1][:, lo_:hi_])

            dma_wia(0)
            if nblk > 2:
                dma_xt(1)
            dma_wia(1)
            w2t = {}
            for m in range(2):
                for t in range(2):
                    w = cpool.tile([128, 2 * 128], fp8, tag=f"w2_{m}{t}")
                    nc.sync.dma_start(w[:], w2_d[m, t])
                    w2t[m, t] = w[:].rearrange("p (k m) -> p k m", k=2)
            b2a = cpool.tile([128, 1], f32, tag="b2a")
            b2b = cpool.tile([128, 1], f32, tag="b2b")
            nc.sync.dma_start(b2a[:], b2_d[0])
            nc.sync.dma_start(b2b[:], b2_d[1])

            out2 = cpool.tile([128, 2 * mc_per_core], f32, tag="out2")

            def l1(k):
                sb, half = divmod(k, 2)
                if half == 0 and sb + 2 < nblk // 2:
                    dma_xt(sb + 2)
                if k % 16 == 0:
                    dma_wia(k // 16 + 2)
                xls = xts[sb][:, half * BLK:(half + 1) * BLK]
                # single 2-bank psum tile; the K=8 indicator matmuls sit
                # adjacent in PE order (distinct row groups -> concurrent)
                # while keeping each bank's accumulation group contiguous
                ps1 = psum.tile([128, 2 * BLK], f32, tag="ps1")
                nc.tensor.matmul(ps1[:, 0:BLK], w1xa[:], xls,
                                 start=True, stop=False)
                nc.tensor.matmul(ps1[:, 0:BLK],
                                 wia[0:8, k * 128:(k + 1) * 128],
                                 xind[0:8, :], start=False, stop=True)
                nc.tensor.matmul(ps1[:, BLK:2 * BLK],
                                 wia[32:40, k * 128:(k + 1) * 128],
                                 xind[32:40, :], start=True, stop=False)
                nc.tensor.matmul(ps1[:, BLK:2 * BLK], w1xb[:], xls,
                                 start=False, stop=True)
                ps1s[k] = ps1
                if half == 1:
                    xts.pop(sb, None)

            def h1f(k):
                ps1 = ps1s.pop(k)
                h1 = pool.tile([128, 2 * BLK], fp8, tag="h1")
                nc.scalar.activation(h1[:], ps1[:], Act.Relu)
                h1s[k] = h1

            def l2(k):
                h1 = h1s.pop(k)
                h1v = h1[:].rearrange("p (t c) -> p t c", t=2)
                lo = k % 2 == 0
                ps2a = psum.tile([128, BLK], f32, tag="ps2a")
                nc.tensor.matmul(ps2a[:], w2t[0, 0], h1v, start=True,
                                 stop=not lo, perf_mode=DR)
                if lo:
                    nc.tensor.matmul(ps2a[:], w2t[0, 1], h1v, start=False,
                                     stop=True, perf_mode=DR)
                ps2b = psum.tile([128, BLK], f32, tag="ps2b")
                nc.tensor.matmul(ps2b[:], w2t[1, 0], h1v, start=True,
                                 stop=not lo, perf_mode=DR)
                if lo:
                    nc.tensor.matmul(ps2b[:], w2t[1, 1], h1v, start=False,
                                     stop=True, perf_mode=DR)
                ps2s[k] = (ps2a, ps2b)

            def h2f(k):
                ps2a, ps2b = ps2s.pop(k)
                h2 = pool.tile([128, 2 * BLK], bf16, tag="h2")
                # half a runs on ACT 3/4 of the time to balance ACT vs DVE
                if k % 4 != 3:
                    nc.scalar.activation(h2[:, 0:BLK], ps2a[:], Act.Relu,
                                         bias=b2a[:])
                else:
                    nc.vector.tensor_scalar(h2[:, 0:BLK], ps2a[:], b2a[:],
                                            0.0, op0=Alu.add, op1=Alu.max)
                nc.vector.tensor_scalar(h2[:, BLK:2 * BLK], ps2b[:], b2b[:],
                                        0.0, op0=Alu.add, op1=Alu.max)
                h2s[k] = h2

            hfs = {}

            def fold1(k):
                h2 = h2s.pop(k)
                h2v = h2[:].rearrange("p (g t m) -> p g t m", t=2, m=MC // 2)
                hf = pool.tile([128, BLK], bf16, tag="hf")
                hfv = hf[:].rearrange("p (g m) -> p g m", m=MC // 2)
                nc.gpsimd.tensor_tensor(
                    hfv, h2v[:, :, 0:1, :], h2v[:, :, 1:2, :], op=Alu.add)
                hfs[k] = hf

            def red(k):
                hf = hfs.pop(k)
                hfv = hf[:].rearrange("p (g m) -> p g m", m=MC // 2)
                nc.vector.tensor_reduce(
                    out2[:, k * 16:(k + 1) * 16], hfv,
                    axis=mybir.AxisListType.X, op=Alu.add)
                # stream finished output chunks out during the loop
                if (k + 1) % 32 == 0:
                    nc.sync.dma_start(out_d[:, (k - 31) * 16:(k + 1) * 16],
                                      out2[:, (k - 31) * 16:(k + 1) * 16])

            # every stage's dependencies are >=1 iteration old, so no engine
            # ever head-blocks its FIFO waiting on same-iteration work
            for k in range(nblk + 5):
                if k < nblk:
                    l1(k)
                if 0 <= k - 2 < nblk:
                    l2(k - 2)
                if 0 <= k - 1 < nblk:
                    h1f(k - 1)
                if 0 <= k - 3 < nblk:
                    h2f(k - 3)
                if 0 <= k - 4 < nblk:
                    fold1(k - 4)
                if 0 <= k - 5 < nblk:
                    red(k - 5)

            tail = (nblk // 32) * 32
            if tail < nblk:
                nc.sync.dma_start(out_d[:, tail * 16:],
                                  out2[:, tail * 16:])

    nc.compile()
    return nc


def _get_program(nblk):
    if nblk not in _compiled:
        _compiled[nblk] = _build_program(nblk)
    return _compiled[nblk]


def kernel(X, Z, W1, b1, W2, b2, W3, b3, cell_to_batch, sample_idx_batch):
    X = np.asarray(X)
    Z = np.asarray(Z)
    W1 = np.asarray(W1, dtype=np.float32)
    b1 = np.asarray(b1, dtype=np.float32)
    W2 = np.asarray(W2, dtype=np.float32)
    b2 = np.asarray(b2, dtype=np.float32)
    W3 = np.asarray(W3, dtype=np.float32)
    b3 = np.asarray(b3, dtype=np.float32)
    c2b = np.asarray(cell_to_batch).astype(np.int64)
    sib = np.asarray(sample_idx_batch).astype(np.int64)

    n = X.shape[0]
    nseg = sib.shape[0]
    seg = sib[c2b]

    # ---- host layout prep -------------------------------------------------
    order = np.argsort(seg, kind="stable")
    seg_sorted = seg[order]
    counts = np.bincount(seg, minlength=nseg).astype(np.int64)
    padded = ((counts + MC - 1) // MC) * MC
    starts = np.concatenate([[0], np.cumsum(padded)])[:nseg]
    total_pad = int(padded.sum())
    nblk = NBLK
    while total_pad > N_CORES * nblk * BLK:  # safety fallback, recompiles
        nblk += 2
    ntot = N_CORES * nblk * BLK
    mc_per_core = nblk * (BLK // MC)
    run_starts = np.concatenate([[0], np.cumsum(counts)])[:nseg]
    ranks = np.arange(n, dtype=np.int64) - run_starts[seg_sorted]
    slots = starts[seg_sorted] + ranks

    Xs = np.zeros((ntot, DX), dtype=BF16)
    Xs[slots] = np.log1p(X[order], dtype=np.float32).astype(BF16)

    xt = np.ascontiguousarray(
        Xs.reshape(N_CORES, nblk // 2, 2 * BLK, DX).transpose(0, 1, 3, 2))

    n_mc = ntot // MC
    mc_label = np.full(n_mc, -1, dtype=np.int64)
    mc_real = np.zeros(n_mc, dtype=np.int64)
    mc_of_slot = slots // MC
    mc_label[mc_of_slot] = seg_sorted
    np.add.at(mc_real, mc_of_slot, 1)

    # ---- weights ----------------------------------------------------------
    w1x = np.ascontiguousarray(W1[:DX]).astype(BF16)
    # per-sample covariate projection, folded with b1; bf16 as shipped
    zb1_bf = (Z.astype(np.float32) @ W1[DX:DX + DZ] + b1).astype(BF16)
    lab = mc_label.reshape(N_CORES, nblk, 8)
    wind = zb1_bf[np.maximum(lab, 0)]              # [C, nblk, 8, 256]
    wind[lab < 0] = 0
    wind = np.ascontiguousarray(
        wind.reshape(N_CORES, nblk, 8, 2, 128)
        .transpose(0, 3, 2, 1, 4)                  # [C, half, g, blk, 128]
        .reshape(N_CORES, 2, 8, nblk * 128))

    xind = np.zeros((40, BLK), dtype=BF16)
    for g in range(BLK // MC):
        xind[g, g * MC:(g + 1) * MC] = 1
        xind[32 + g, g * MC:(g + 1) * MC] = 1

    # W2 as a scaled fp8 (hi, lo) pair; together they are W2 to ~4e-4
    w2f = W2.astype(BF16).astype(np.float32) * W2SCALE
    t_hi = w2f.astype(FP8)
    # lo term ships pre-doubled: it is applied on even blocks only
    t_lo = (2.0 * (w2f - t_hi.astype(np.float32))).astype(FP8)
    w2q = np.zeros((2, 2, 128, 2 * 128), dtype=FP8)
    for m in range(2):
        for t, term in enumerate((t_hi, t_lo)):
            # [p, ktile*128] with element [p, k*128+mc] = term[k*128+p, m*128+mc]
            w2q[m, t] = (term.reshape(2, 128, H).transpose(1, 0, 2)
                         [:, :, m * 128:(m + 1) * 128].reshape(128, 256))
    b2d = np.ascontiguousarray(b2.reshape(2, 128, 1)) * W2SCALE

    # ---- run on 8 cores ---------------------------------------------------
    nc = _get_program(nblk)
    in_maps = []
    for c in range(N_CORES):
        in_maps.append({
            "xt": xt[c], "wind": wind[c], "xind": xind,
            "w1x": w1x, "w2": w2q, "b2": b2d,
        })
    global _last_in_maps
    _last_in_maps = in_maps
    res = run_bass_kernel_spmd(nc, in_maps, list(range(N_CORES)))

    # ---- host epilogue ----------------------------------------------------
    per_core = []
    for c in range(N_CORES):
        o = res.results[c]["out"].reshape(128, nblk, 2, BLK // MC)
        per_core.append(np.concatenate(
            [o[:, :, 0, :].reshape(128, mc_per_core),
             o[:, :, 1, :].reshape(128, mc_per_core)], axis=0))
    sums = np.concatenate(per_core, axis=1)  # [256, n_mc], scaled by W2SCALE

    # analytic contribution of one pad cell (X'=0, zb1 applied), matching
    # device math; every 4th block includes the 4x lo-term, others hi-only
    h1p = np.maximum(zb1_bf.astype(np.float32), 0.0) \
        .astype(FP8).astype(np.float32)                      # [B, 256]
    w2eff = t_hi.astype(np.float32) + t_lo.astype(np.float32)
    v_even = np.maximum(h1p @ w2eff + W2SCALE * b2, 0.0) \
        .astype(BF16).astype(np.float32)                     # [B, 256]
    v_odd = np.maximum(h1p @ t_hi.astype(np.float32) + W2SCALE * b2, 0.0) \
        .astype(BF16).astype(np.float32)
    mc_parity = ((np.arange(n_mc) // (BLK // MC)) % nblk) % 2
    npad = MC - mc_real
    fix = (mc_label >= 0) & (npad > 0)
    vp = np.where(mc_parity[fix, None] == 0,
                  v_even[mc_label[fix]], v_odd[mc_label[fix]])
    sums[:, fix] -= (vp * npad[fix, None].astype(np.float32)).T
    sums /= W2SCALE

    valid = mc_label >= 0
    S = np.zeros((nseg, H), dtype=np.float32)
    np.add.at(S, mc_label[valid], sums[:, valid].T)

    denom = np.maximum(counts, 1).astype(np.float32)[:, None]
    Y = S @ W3 / denom + b3[None, :]
    Y[counts == 0] = 0.0
    return Y.astype(np.float32)
